# revision 62
# baseline (speedup 1.0000x reference)
"""Trainium2 Bass kernel for nn_GridToMeshEncoder.

Computes: bilinear 4-corner gather from a (B,721,1440,64) grid at 40962 mesh
nodes + weighted corner sum, concat 4 mesh features, 2-layer MLP (68->256->256).

Strategy (v5): mesh nodes sharded across 8 NeuronCores (5248 padded nodes per
core, both batches on every core). The irregular corner gather runs on the
host (TRN2 indirect DMA is descriptor-rate-limited for 256B rows — measured
4x slower than the dense-DMA floor), packed into the exact partition-major
bf16 tile layout the device consumes. The device pipeline is bf16 end-to-end
(PSUM accumulation in fp32), spread across all five engine queues so each
stays under ~50% busy: corner-weight multiply on Pool, 4-corner reduction on
DVE, per-tile PE transposes (x and mesh-feature tiles) feed a W1 matmul with
nodes streaming on the free dim at N=512, relu+bias on ACT, and the second
layer computed transposed (y_t = W2q^T @ h) so the four W2 quadrants are the
stationary operands. PSUM->SBUF drains alternate between ACT and DVE per
chunk; output stores alternate between the SP (HWDGE) and Pool (SWDGE) DMA
queues. The device emits y transposed in bf16; the host widens to fp32 and
un-transposes while assembling the full output (exact widening — the only
loss is bf16 rounding, rel err ~4.5e-3, well inside the 2e-2 gate).

Self-contained: hardcodes all shapes; imports bass from /opt/trn_rl_repo.
"""

import sys
from dataclasses import dataclass

import numpy as np

_TRN_REPO = "/opt/trn_rl_repo"
if _TRN_REPO not in sys.path:
    sys.path.insert(0, _TRN_REPO)

import concourse.mybir as mybir  # noqa: E402
import concourse.tile as tile  # noqa: E402
from concourse import bacc  # noqa: E402
from concourse.masks import make_identity  # noqa: E402

# Problem constants
B = 2
N_LAT, N_LON = 721, 1440
G = N_LAT * N_LON  # 1038240 grid rows per batch
C = 64  # grid channels
M = 40962  # mesh nodes
F = 4  # mesh features
DIN = C + F  # 68
HID = 256
OUT = 256
N_CORES = 8

BF16 = mybir.dt.bfloat16
NPDT = mybir.dt.np(BF16)


@dataclass(frozen=True)
class Cfg:
    npc: int = 5248  # nodes per core (41 tiles of 128)
    add_b2: bool = False
    add_b1: bool = False
    loop_k: int = 0  # >0: wrap compute in a hardware loop (timing builds)
    bufs: tuple = (4, 3, 3, 3, 3, 3)  # gp, spool, xp, xtp, htp, yp
    mult_on_dve: bool = False  # weighted-corner multiply on DVE vs Pool
    sc: int = 4  # superchunk size in tiles (load/mult/reduce granularity)
    stores_split: bool = True  # out-DMAs alternate SP/Pool vs all on Pool
    relu_split: bool = False  # relu h=1 on DVE (tensor_scalar) vs both ACT
    ps_bufs: tuple = (2, 3, 3)  # ps_xt, ps_ht, ps_y (<= 8 banks total)
    merged_y: bool = False  # y PSUM as one [128,1024] tile, single drain op
    staggered: bool = False  # For_i staggered_reset (no per-iter barrier)
    # 0: k-reduce on DVE always; 1: odd chunks reduce on PE via
    # accumulating transposes; 2: always on PE
    pe_reduce: int = 0
    merged_ht: bool = False  # ht PSUM as one [128,1024] tile, single relu op
    pool_reduce: bool = False  # k-reduce as 3 tensor adds on Pool
    # timing-ablation switches (correctness only valid when all True)
    do_load: bool = True
    do_mult: bool = True
    do_reduce: bool = True
    do_mm: bool = True
    do_store: bool = True

    @property
    def tiles(self):
        assert self.npc % 128 == 0
        return self.npc // 128

    @property
    def chunk_plan(self):
        plan, t = [], 0
        while t < self.tiles:
            k = min(self.sc, self.tiles - t)
            plan.append((t, k))
            t += k
        return plan

    @property
    def n_chunks(self):
        return len(self.chunk_plan)


def build_nc(cfg: Cfg):
    """Build the per-core Bass program (identical across all 8 cores)."""
    f32 = mybir.dt.float32
    nc = bacc.Bacc("TRN2", target_bir_lowering=False, debug=False)
    T = cfg.tiles
    NCH = cfg.n_chunks

    # host-gathered corners, bf16: [b, chunk, p, t*256 + k*64 + c]
    gc_d = nc.dram_tensor("gcorn", [B, NCH, 128, cfg.sc * 256], BF16,
                          kind="ExternalInput")
    w_d = nc.dram_tensor("wts", [128, T * 4], BF16, kind="ExternalInput")
    mf_d = nc.dram_tensor("mf", [128, T * F], BF16, kind="ExternalInput")
    w1_d = nc.dram_tensor("W1", [DIN, HID], BF16, kind="ExternalInput")
    b1_d = nc.dram_tensor("b1r", [128, 2], f32, kind="ExternalInput")
    # W2 quadrants: [hidhalf*2+outhalf, 128 hid, 128 out]
    w2_d = nc.dram_tensor("W2q", [4, 128, 128], BF16, kind="ExternalInput")
    if cfg.add_b2:
        b2_d = nc.dram_tensor("b2r", [128, 2], f32, kind="ExternalInput")
    # output transposed: [outhalf, outch(128), b*npc + node]
    out_d = nc.dram_tensor("out", [2, 128, B * cfg.npc], BF16,
                           kind="ExternalOutput")

    with tile.TileContext(nc) as tc:
        with (
            tc.tile_pool(name="res", bufs=1) as res,
            tc.tile_pool(name="gp", bufs=cfg.bufs[0]) as gp,
            tc.tile_pool(name="sp", bufs=cfg.bufs[1]) as spool,
            tc.tile_pool(name="xp", bufs=cfg.bufs[2]) as xp,
            tc.tile_pool(name="rp", bufs=4) as rp,
            tc.tile_pool(name="xtp", bufs=cfg.bufs[3]) as xtp,
            tc.tile_pool(name="htp", bufs=cfg.bufs[4]) as htp,
            tc.tile_pool(name="yp", bufs=cfg.bufs[5]) as yp,
            tc.tile_pool(name="ps_xt", bufs=cfg.ps_bufs[0],
                         space="PSUM") as ps_xt,
            tc.tile_pool(name="ps_ht", bufs=cfg.ps_bufs[1],
                         space="PSUM") as ps_ht,
            tc.tile_pool(name="ps_y", bufs=cfg.ps_bufs[2],
                         space="PSUM") as ps_y,
        ):
            w_sb = res.tile([128, T * 4], BF16)
            mf_sb = res.tile([128, T * F], BF16)
            w1_sb = res.tile([DIN, HID], BF16)
            b1_sb = res.tile([128, 2], f32)
            w2_sb = res.tile([128, 4 * 128], BF16)
            ident = res.tile([128, 128], BF16)

            nc.sync.dma_start(out=w_sb[:], in_=w_d[:])
            nc.sync.dma_start(out=mf_sb[:], in_=mf_d[:])
            nc.sync.dma_start(out=w1_sb[:], in_=w1_d[:])
            nc.sync.dma_start(out=b1_sb[:], in_=b1_d[:])
            for q in range(4):
                nc.sync.dma_start(out=w2_sb[:, q * 128:(q + 1) * 128],
                                  in_=w2_d[q])
            if cfg.add_b2:
                b2_sb = res.tile([128, 2], f32)
                nc.sync.dma_start(out=b2_sb[:], in_=b2_d[:])
            make_identity(nc, ident[:])

            def mm_group(gi, src, xoff, nbase, tg0, gt, pe_red=False):
                """4-tile MM group: transpose -> W1 -> relu -> W2 -> store.

                pe_red: src is `scaled` [128, sc*256]; the 4-corner sum runs
                on the PE as 4 accumulating transposes per tile. Otherwise
                src is the DVE-reduced x [128, sc*64], one transpose per tile.
                """
                nn = gt * 128
                xt_ps = ps_xt.tile([DIN, 4 * 128], BF16, tag="xtps")
                for tl in range(gt):
                    if pe_red:
                        for k in range(4):
                            col = ((xoff + tl) * 4 + k) * 64
                            nc.tensor.matmul(
                                out=xt_ps[0:64, tl * 128:(tl + 1) * 128],
                                lhsT=src[:, col:col + 64],
                                rhs=ident[:],
                                is_transpose=True,
                                start=(k == 0), stop=(k == 3),
                            )
                    else:
                        nc.tensor.transpose(
                            out=xt_ps[0:64, tl * 128:(tl + 1) * 128],
                            in_=src[:, (xoff + tl) * 64:(xoff + tl + 1) * 64],
                            identity=ident[:],
                        )
                    nc.tensor.transpose(
                        out=xt_ps[64:68, tl * 128:(tl + 1) * 128],
                        in_=mf_sb[:, (tg0 + tl) * 4:(tg0 + tl + 1) * 4],
                        identity=ident[:],
                    )
                xt = xtp.tile([DIN, 4 * 128], BF16, tag="xt")
                if gi % 2 == 0 or cfg.relu_split:
                    nc.scalar.activation(
                        out=xt[:, :nn], in_=xt_ps[:, :nn],
                        func=mybir.ActivationFunctionType.Copy)
                else:
                    nc.vector.tensor_copy(out=xt[:, :nn], in_=xt_ps[:, :nn])
                # --- layer 1: ht[h, n] = W1h^T @ xt ---
                ht = htp.tile([128, 2 * 512], BF16, tag="ht")
                if cfg.merged_ht and not cfg.add_b1:
                    ht_ps = ps_ht.tile([128, 1024], f32, tag="htps")
                    for h in range(2):
                        nc.tensor.matmul(
                            out=ht_ps[:, h * 512: h * 512 + nn],
                            lhsT=w1_sb[:, h * 128:(h + 1) * 128],
                            rhs=xt[:, :nn], start=True, stop=True,
                        )
                    # single relu over both hid halves (b1 known zero)
                    nc.scalar.activation(
                        out=ht[:].rearrange("p (o n) -> p o n",
                                            o=2)[:, :, :nn],
                        in_=ht_ps[:].rearrange("p (o n) -> p o n",
                                               o=2)[:, :, :nn],
                        func=mybir.ActivationFunctionType.Relu,
                    )
                else:
                    for h in range(2):
                        ht_ps = ps_ht.tile([128, 512], f32, tag="htps")
                        nc.tensor.matmul(
                            out=ht_ps[:, :nn],
                            lhsT=w1_sb[:, h * 128:(h + 1) * 128],
                            rhs=xt[:, :nn], start=True, stop=True,
                        )
                        if cfg.relu_split and h == 1:
                            nc.vector.tensor_scalar(
                                out=ht[:, h * 512: h * 512 + nn],
                                in0=ht_ps[:, :nn],
                                scalar1=b1_sb[:, h:h + 1], scalar2=0.0,
                                op0=mybir.AluOpType.add,
                                op1=mybir.AluOpType.max,
                            )
                        else:
                            nc.scalar.activation(
                                out=ht[:, h * 512: h * 512 + nn],
                                in_=ht_ps[:, :nn],
                                func=mybir.ActivationFunctionType.Relu,
                                bias=b1_sb[:, h:h + 1], scale=1.0,
                            )
                # --- layer 2 transposed: y[o, n] = sum_h W2q^T @ ht ---
                y = yp.tile([128, 2 * 512], BF16, tag="y")
                if cfg.merged_y and not cfg.add_b2:
                    y_ps = ps_y.tile([128, 1024], f32, tag="yps")
                    for oh in range(2):
                        for hh in range(2):
                            nc.tensor.matmul(
                                out=y_ps[:, oh * 512: oh * 512 + nn],
                                lhsT=w2_sb[:, (hh * 2 + oh) * 128:
                                           (hh * 2 + oh + 1) * 128],
                                rhs=ht[:, hh * 512: hh * 512 + nn],
                                start=(hh == 0), stop=(hh == 1),
                            )
                    yv = y[:].rearrange("p (o n) -> p o n", o=2)[:, :, :nn]
                    ypv = y_ps[:].rearrange("p (o n) -> p o n",
                                            o=2)[:, :, :nn]
                    if gi % 2 == 0:
                        nc.scalar.activation(
                            out=yv, in_=ypv,
                            func=mybir.ActivationFunctionType.Copy)
                    else:
                        nc.vector.tensor_copy(out=yv, in_=ypv)
                else:
                    for oh in range(2):
                        y_ps = ps_y.tile([128, 512], f32, tag="yps")
                        for hh in range(2):
                            nc.tensor.matmul(
                                out=y_ps[:, :nn],
                                lhsT=w2_sb[:, (hh * 2 + oh) * 128:
                                           (hh * 2 + oh + 1) * 128],
                                rhs=ht[:, hh * 512: hh * 512 + nn],
                                start=(hh == 0), stop=(hh == 1),
                            )
                        # (oh ^ gi%2) alternation keeps DVE/ACT evenly fed
                        if cfg.add_b2:
                            nc.scalar.activation(
                                out=y[:, oh * 512: oh * 512 + nn],
                                in_=y_ps[:, :nn],
                                func=mybir.ActivationFunctionType.Identity,
                                bias=b2_sb[:, oh:oh + 1], scale=1.0,
                            )
                        elif (oh + gi) % 2 == 0:
                            nc.scalar.activation(
                                out=y[:, oh * 512: oh * 512 + nn],
                                in_=y_ps[:, :nn],
                                func=mybir.ActivationFunctionType.Copy,
                            )
                        else:
                            nc.vector.tensor_copy(
                                out=y[:, oh * 512: oh * 512 + nn],
                                in_=y_ps[:, :nn],
                            )
                if cfg.do_store:
                    n0 = nbase + tg0 * 128
                    eng0 = nc.sync if cfg.stores_split else nc.gpsimd
                    eng0.dma_start(out=out_d[0, :, n0:n0 + nn],
                                   in_=y[:, 0:nn])
                    nc.gpsimd.dma_start(out=out_d[1, :, n0:n0 + nn],
                                        in_=y[:, 512: 512 + nn])

            def body():
                gi = 0
                for b in range(B):
                    for ci, (t0, kt) in enumerate(cfg.chunk_plan):
                        # --- dense load of host-gathered corners (bf16) ---
                        if cfg.do_load or cfg.do_mult:
                            g = gp.tile([128, cfg.sc * 256], BF16, tag="g")
                        if cfg.do_load:
                            nc.sync.dma_start(out=g[:, :kt * 256],
                                              in_=gc_d[b, ci, :, :kt * 256])
                        elif cfg.do_mult:
                            nc.gpsimd.memset(g[:, :kt * 256], 0.25)
                        # --- weighted corners: scaled = g * w ---
                        if cfg.do_mult or cfg.do_reduce:
                            scaled = spool.tile([128, cfg.sc * 256], BF16,
                                                tag="s")
                        if cfg.do_mult:
                            g_v = g[:, :kt * 256].rearrange(
                                "p (t k c) -> p t k c", k=4, c=64)
                            w_v = (
                                w_sb[:, t0 * 4:(t0 + kt) * 4]
                                .rearrange("p (t k o) -> p t k o", k=4, o=1)
                                .to_broadcast([128, kt, 4, 64])
                            )
                            s_v = scaled[:, :kt * 256].rearrange(
                                "p (t k c) -> p t k c", k=4, c=64)
                            mult_eng = (nc.vector if cfg.mult_on_dve
                                        else nc.gpsimd)
                            mult_eng.tensor_tensor(out=s_v, in0=g_v, in1=w_v,
                                                   op=mybir.AluOpType.mult)
                        elif cfg.do_reduce:
                            nc.gpsimd.memset(scaled[:, :kt * 256], 0.25)
                        # --- corner sum -> x [128, kt*64] (bf16) ---
                        pe_red = cfg.pe_reduce == 2 or (
                            cfg.pe_reduce == 1 and ci % 2 == 1)
                        if pe_red:
                            src = scaled
                        else:
                            x = xp.tile([128, cfg.sc * 64], BF16, tag="x")
                            src = x
                            if not cfg.do_reduce and cfg.do_mm:
                                nc.gpsimd.memset(x[:, :kt * 64], 0.25)
                            if cfg.do_reduce and cfg.pool_reduce:
                                s4 = scaled[:, :kt * 256].rearrange(
                                    "p (t k c) -> p t k c", k=4, c=64)
                                ra = rp.tile([128, cfg.sc * 64], BF16,
                                             tag="ra")
                                rb = rp.tile([128, cfg.sc * 64], BF16,
                                             tag="rb")
                                ra_v = ra[:, :kt * 64].rearrange(
                                    "p (t c) -> p t c", c=64)
                                rb_v = rb[:, :kt * 64].rearrange(
                                    "p (t c) -> p t c", c=64)
                                x_v = x[:, :kt * 64].rearrange(
                                    "p (t c) -> p t c", c=64)
                                nc.gpsimd.tensor_tensor(
                                    out=ra_v, in0=s4[:, :, 0, :],
                                    in1=s4[:, :, 1, :],
                                    op=mybir.AluOpType.add)
                                nc.gpsimd.tensor_tensor(
                                    out=rb_v, in0=s4[:, :, 2, :],
                                    in1=s4[:, :, 3, :],
                                    op=mybir.AluOpType.add)
                                nc.gpsimd.tensor_tensor(
                                    out=x_v, in0=ra_v, in1=rb_v,
                                    op=mybir.AluOpType.add)
                            elif cfg.do_reduce:
                                with nc.allow_low_precision(
                                        reason="4-term bf16 sum, tol 2e-2"):
                                    nc.vector.tensor_reduce(
                                        out=x[:, :kt * 64].rearrange(
                                            "p (t c) -> p t c", c=64),
                                        in_=scaled[:, :kt * 256].rearrange(
                                            "p (t k c) -> p t c k",
                                            k=4, c=64),
                                        axis=mybir.AxisListType.X,
                                        op=mybir.AluOpType.add,
                                    )
                        if cfg.do_mm:
                            for g0 in range(0, kt, 4):
                                gt = min(4, kt - g0)
                                mm_group(gi, src, g0, b * cfg.npc,
                                         t0 + g0, gt, pe_red)
                                gi += 1

            if cfg.loop_k > 0:
                with tc.For_i(0, cfg.loop_k, 1,
                              staggered_reset=cfg.staggered):
                    body()
            else:
                body()
    nc.compile()
    return nc


# ---------------------------------------------------------------------------
# Host side
# ---------------------------------------------------------------------------

_NC_CACHE = {}


def _get_nc(cfg: Cfg):
    key = (cfg.add_b2, cfg.npc, cfg.loop_k)
    if key not in _NC_CACHE:
        _NC_CACHE[key] = build_nc(cfg)
    return _NC_CACHE[key]


def _core_layout(arr, npc, core, width):
    """arr: (M_pad, width) -> per-core [128, tiles*width] partition-major."""
    t = npc // 128
    a = arr[core * npc:(core + 1) * npc]
    return np.ascontiguousarray(
        a.reshape(t, 128, width).transpose(1, 0, 2).reshape(128, t * width)
    )


def make_in_maps(grid_data, mesh_features, indices, weights, W1, b1, W2, b2,
                 cfg):
    grid_data = np.asarray(grid_data, dtype=np.float32)
    mesh_features = np.asarray(mesh_features, dtype=np.float32)
    indices = np.asarray(indices).astype(np.int64)
    weights = np.asarray(weights, dtype=np.float32)
    npc = cfg.npc
    m_pad = N_CORES * npc
    T = cfg.tiles

    grid2d = grid_data.reshape(B * G, C).astype(NPDT)

    wp = np.zeros((m_pad, 4), dtype=np.float32)
    wp[:M] = weights
    mfp = np.zeros((m_pad, F), dtype=np.float32)
    mfp[:M] = mesh_features
    idxp = np.zeros((m_pad, 4), dtype=np.int64)
    idxp[:M] = indices

    b1r = np.ascontiguousarray(np.asarray(b1, np.float32).reshape(2, 128).T)
    # W2 quadrants [hh*2+oh, 128, 128]
    w2 = np.asarray(W2, np.float32)
    w2q = np.stack([w2[hh * 128:(hh + 1) * 128, oh * 128:(oh + 1) * 128]
                    for hh in range(2) for oh in range(2)]).astype(NPDT)
    b2r = np.ascontiguousarray(
        np.asarray(b2, np.float32).reshape(2, 128).T)

    in_maps = []
    for c in range(N_CORES):
        idx_c = idxp[c * npc:(c + 1) * npc]  # (npc, 4)
        gcorn = np.zeros((B, cfg.n_chunks, 128, cfg.sc * 256), dtype=NPDT)
        for b in range(B):
            # (npc, 4, C) -> tiles (T,128,4,C) -> (128, T, 4*C)
            g4 = grid2d[b * G + idx_c]
            g4 = g4.reshape(T, 128, 4 * C).transpose(1, 0, 2)
            for ci, (t0, kt) in enumerate(cfg.chunk_plan):
                gcorn[b, ci, :, :kt * 256] = (
                    g4[:, t0:t0 + kt].reshape(128, kt * 256))
        im = {
            "gcorn": gcorn,
            "wts": _core_layout(wp, npc, c, 4).astype(NPDT),
            "mf": _core_layout(mfp, npc, c, F).astype(NPDT),
            "W1": np.asarray(W1, np.float32).astype(NPDT),
            "b1r": b1r,
            "W2q": w2q,
        }
        if cfg.add_b2:
            im["b2r"] = b2r
        in_maps.append(im)
    return in_maps


def kernel(grid_data, mesh_features, indices, weights, W1, b1, W2, b2):
    cfg = Cfg(add_b2=bool(np.any(np.asarray(b2))),
              add_b1=bool(np.any(np.asarray(b1))))
    nc = _get_nc(cfg)
    in_maps = make_in_maps(grid_data, mesh_features, indices, weights,
                           W1, b1, W2, b2, cfg)

    from concourse.bass_utils import run_bass_kernel_spmd
    res = run_bass_kernel_spmd(nc, in_maps, core_ids=list(range(N_CORES)))

    npc = cfg.npc
    # per-core out: [2(outhalf), 128, B*npc] bf16, nodes ordered [b, node]
    shards = []
    for c in range(N_CORES):
        o = np.asarray(res.results[c]["out"])  # (2, 128, B*npc)
        o = o.reshape(2, 128, B, npc).transpose(2, 3, 0, 1)  # (B,npc,2,128)
        shards.append(o.reshape(B, npc, OUT))
    y = np.concatenate(shards, axis=1)[:, :M, :].astype(np.float32)
    return np.ascontiguousarray(y)


# revision 67
# speedup vs baseline: 1.0102x; 1.0102x over previous
"""Trainium2 Bass kernel for nn_GridToMeshEncoder.

Computes: bilinear 4-corner gather from a (B,721,1440,64) grid at 40962 mesh
nodes + weighted corner sum, concat 4 mesh features, 2-layer MLP (68->256->256).

Strategy (v5): mesh nodes sharded across 8 NeuronCores (5248 padded nodes per
core, both batches on every core). The irregular corner gather runs on the
host (TRN2 indirect DMA is descriptor-rate-limited for 256B rows — measured
4x slower than the dense-DMA floor), packed into the exact partition-major
bf16 tile layout the device consumes. The device pipeline is bf16 end-to-end
(PSUM accumulation in fp32), spread across all five engine queues so each
stays under ~50% busy: corner-weight multiply on Pool, 4-corner reduction on
DVE, per-tile PE transposes (x and mesh-feature tiles) feed a W1 matmul with
nodes streaming on the free dim at N=512, relu+bias on ACT, and the second
layer computed transposed (y_t = W2q^T @ h) so the four W2 quadrants are the
stationary operands. PSUM->SBUF drains alternate between ACT and DVE per
chunk; output stores alternate between the SP (HWDGE) and Pool (SWDGE) DMA
queues. The device emits y transposed in bf16; the host widens to fp32 and
un-transposes while assembling the full output (exact widening — the only
loss is bf16 rounding, rel err ~4.5e-3, well inside the 2e-2 gate).

Self-contained: hardcodes all shapes; imports bass from /opt/trn_rl_repo.
"""

import sys
from dataclasses import dataclass

import numpy as np

_TRN_REPO = "/opt/trn_rl_repo"
if _TRN_REPO not in sys.path:
    sys.path.insert(0, _TRN_REPO)

import concourse.mybir as mybir  # noqa: E402
import concourse.tile as tile  # noqa: E402
from concourse import bacc  # noqa: E402
from concourse.masks import make_identity  # noqa: E402

# Problem constants
B = 2
N_LAT, N_LON = 721, 1440
G = N_LAT * N_LON  # 1038240 grid rows per batch
C = 64  # grid channels
M = 40962  # mesh nodes
F = 4  # mesh features
DIN = C + F  # 68
HID = 256
OUT = 256
N_CORES = 8

BF16 = mybir.dt.bfloat16
NPDT = mybir.dt.np(BF16)


@dataclass(frozen=True)
class Cfg:
    npc: int = 5248  # nodes per core (41 tiles of 128)
    add_b2: bool = False
    add_b1: bool = False
    loop_k: int = 0  # >0: wrap compute in a hardware loop (timing builds)
    bufs: tuple = (4, 3, 3, 3, 3, 3)  # gp, spool, xp, xtp, htp, yp
    mult_on_dve: bool = False  # weighted-corner multiply on DVE vs Pool
    sc: int = 4  # superchunk size in tiles (load/mult/reduce granularity)
    taper: int = 0  # 0: none; 1: 1,2,4..4,2 chunk plan; 2: 2,4..4,3
    stores_split: bool = True  # out-DMAs alternate SP/Pool vs all on Pool
    relu_split: bool = False  # relu h=1 on DVE (tensor_scalar) vs both ACT
    ps_bufs: tuple = (2, 3, 3)  # ps_xt, ps_ht, ps_y (<= 8 banks total)
    merged_y: bool = False  # y PSUM as one [128,1024] tile, single drain op
    staggered: bool = False  # For_i staggered_reset (no per-iter barrier)
    # 0: k-reduce on DVE always; 1: odd chunks reduce on PE via
    # accumulating transposes; 2: always on PE
    pe_reduce: int = 0
    merged_ht: bool = False  # ht PSUM as one [128,1024] tile, single relu op
    pool_reduce: bool = False  # k-reduce as 3 tensor adds on Pool
    # timing-ablation switches (correctness only valid when all True)
    do_load: bool = True
    do_mult: bool = True
    do_reduce: bool = True
    do_mm: bool = True
    do_store: bool = True

    @property
    def tiles(self):
        assert self.npc % 128 == 0
        return self.npc // 128

    def _plan_from_widths(self, widths):
        if widths is None or sum(widths) != self.tiles:
            widths = None
        plan, t = [], 0
        if widths is not None:
            for k in widths:
                plan.append((t, k))
                t += k
            return plan
        while t < self.tiles:
            k = min(self.sc, self.tiles - t)
            plan.append((t, k))
            t += k
        return plan

    def chunk_plan_for(self, b):
        # optional taper: narrow first (and last) chunks prime/drain the
        # cross-engine pipeline faster than full-width ones
        full = (self.tiles - 5) // 4
        if self.taper == 1 and self.sc == 4:
            widths = [1, 2] + [4] * full + [2]
        elif self.taper == 2 and self.sc == 4:
            widths = [2] + [4] * full + [3]
        elif self.taper == 3 and self.sc == 4:
            # taper only at body start (b=0) and body end (b=B-1)
            if b == 0:
                widths = [1, 2] + [4] * full + [2]
            else:
                widths = [2] + [4] * full + [2, 1]
        else:
            widths = None
        return self._plan_from_widths(widths)

    @property
    def chunk_plan(self):
        return self.chunk_plan_for(0)

    @property
    def n_chunks(self):
        return len(self.chunk_plan)


def build_nc(cfg: Cfg):
    """Build the per-core Bass program (identical across all 8 cores)."""
    f32 = mybir.dt.float32
    nc = bacc.Bacc("TRN2", target_bir_lowering=False, debug=False)
    T = cfg.tiles
    NCH = cfg.n_chunks

    # host-gathered corners, bf16: [b, chunk, p, t*256 + k*64 + c]
    gc_d = nc.dram_tensor("gcorn", [B, NCH, 128, cfg.sc * 256], BF16,
                          kind="ExternalInput")
    w_d = nc.dram_tensor("wts", [128, T * 4], BF16, kind="ExternalInput")
    mf_d = nc.dram_tensor("mf", [128, T * F], BF16, kind="ExternalInput")
    w1_d = nc.dram_tensor("W1", [DIN, HID], BF16, kind="ExternalInput")
    b1_d = nc.dram_tensor("b1r", [128, 2], f32, kind="ExternalInput")
    # W2 quadrants: [hidhalf*2+outhalf, 128 hid, 128 out]
    w2_d = nc.dram_tensor("W2q", [4, 128, 128], BF16, kind="ExternalInput")
    if cfg.add_b2:
        b2_d = nc.dram_tensor("b2r", [128, 2], f32, kind="ExternalInput")
    # output transposed: [outhalf, outch(128), b*npc + node]
    out_d = nc.dram_tensor("out", [2, 128, B * cfg.npc], BF16,
                           kind="ExternalOutput")

    with tile.TileContext(nc) as tc:
        with (
            tc.tile_pool(name="res", bufs=1) as res,
            tc.tile_pool(name="gp", bufs=cfg.bufs[0]) as gp,
            tc.tile_pool(name="sp", bufs=cfg.bufs[1]) as spool,
            tc.tile_pool(name="xp", bufs=cfg.bufs[2]) as xp,
            tc.tile_pool(name="rp", bufs=4) as rp,
            tc.tile_pool(name="xtp", bufs=cfg.bufs[3]) as xtp,
            tc.tile_pool(name="htp", bufs=cfg.bufs[4]) as htp,
            tc.tile_pool(name="yp", bufs=cfg.bufs[5]) as yp,
            tc.tile_pool(name="ps_xt", bufs=cfg.ps_bufs[0],
                         space="PSUM") as ps_xt,
            tc.tile_pool(name="ps_ht", bufs=cfg.ps_bufs[1],
                         space="PSUM") as ps_ht,
            tc.tile_pool(name="ps_y", bufs=cfg.ps_bufs[2],
                         space="PSUM") as ps_y,
        ):
            w_sb = res.tile([128, T * 4], BF16)
            mf_sb = res.tile([128, T * F], BF16)
            w1_sb = res.tile([DIN, HID], BF16)
            b1_sb = res.tile([128, 2], f32)
            w2_sb = res.tile([128, 4 * 128], BF16)
            ident = res.tile([128, 128], BF16)

            nc.sync.dma_start(out=w_sb[:], in_=w_d[:])
            nc.sync.dma_start(out=mf_sb[:], in_=mf_d[:])
            nc.sync.dma_start(out=w1_sb[:], in_=w1_d[:])
            nc.sync.dma_start(out=b1_sb[:], in_=b1_d[:])
            for q in range(4):
                nc.sync.dma_start(out=w2_sb[:, q * 128:(q + 1) * 128],
                                  in_=w2_d[q])
            if cfg.add_b2:
                b2_sb = res.tile([128, 2], f32)
                nc.sync.dma_start(out=b2_sb[:], in_=b2_d[:])
            make_identity(nc, ident[:])

            def mm_group(gi, src, xoff, nbase, tg0, gt, pe_red=False):
                """4-tile MM group: transpose -> W1 -> relu -> W2 -> store.

                pe_red: src is `scaled` [128, sc*256]; the 4-corner sum runs
                on the PE as 4 accumulating transposes per tile. Otherwise
                src is the DVE-reduced x [128, sc*64], one transpose per tile.
                """
                nn = gt * 128
                xt_ps = ps_xt.tile([DIN, 4 * 128], BF16, tag="xtps")
                for tl in range(gt):
                    if pe_red:
                        for k in range(4):
                            col = ((xoff + tl) * 4 + k) * 64
                            nc.tensor.matmul(
                                out=xt_ps[0:64, tl * 128:(tl + 1) * 128],
                                lhsT=src[:, col:col + 64],
                                rhs=ident[:],
                                is_transpose=True,
                                start=(k == 0), stop=(k == 3),
                            )
                    else:
                        nc.tensor.transpose(
                            out=xt_ps[0:64, tl * 128:(tl + 1) * 128],
                            in_=src[:, (xoff + tl) * 64:(xoff + tl + 1) * 64],
                            identity=ident[:],
                        )
                    nc.tensor.transpose(
                        out=xt_ps[64:68, tl * 128:(tl + 1) * 128],
                        in_=mf_sb[:, (tg0 + tl) * 4:(tg0 + tl + 1) * 4],
                        identity=ident[:],
                    )
                xt = xtp.tile([DIN, 4 * 128], BF16, tag="xt")
                if gi % 2 == 0 or cfg.relu_split:
                    nc.scalar.activation(
                        out=xt[:, :nn], in_=xt_ps[:, :nn],
                        func=mybir.ActivationFunctionType.Copy)
                else:
                    nc.vector.tensor_copy(out=xt[:, :nn], in_=xt_ps[:, :nn])
                # --- layer 1: ht[h, n] = W1h^T @ xt ---
                ht = htp.tile([128, 2 * 512], BF16, tag="ht")
                if cfg.merged_ht and not cfg.add_b1:
                    ht_ps = ps_ht.tile([128, 1024], f32, tag="htps")
                    for h in range(2):
                        nc.tensor.matmul(
                            out=ht_ps[:, h * 512: h * 512 + nn],
                            lhsT=w1_sb[:, h * 128:(h + 1) * 128],
                            rhs=xt[:, :nn], start=True, stop=True,
                        )
                    # single relu over both hid halves (b1 known zero)
                    nc.scalar.activation(
                        out=ht[:].rearrange("p (o n) -> p o n",
                                            o=2)[:, :, :nn],
                        in_=ht_ps[:].rearrange("p (o n) -> p o n",
                                               o=2)[:, :, :nn],
                        func=mybir.ActivationFunctionType.Relu,
                    )
                else:
                    for h in range(2):
                        ht_ps = ps_ht.tile([128, 512], f32, tag="htps")
                        nc.tensor.matmul(
                            out=ht_ps[:, :nn],
                            lhsT=w1_sb[:, h * 128:(h + 1) * 128],
                            rhs=xt[:, :nn], start=True, stop=True,
                        )
                        if cfg.relu_split and h == 1:
                            nc.vector.tensor_scalar(
                                out=ht[:, h * 512: h * 512 + nn],
                                in0=ht_ps[:, :nn],
                                scalar1=b1_sb[:, h:h + 1], scalar2=0.0,
                                op0=mybir.AluOpType.add,
                                op1=mybir.AluOpType.max,
                            )
                        else:
                            nc.scalar.activation(
                                out=ht[:, h * 512: h * 512 + nn],
                                in_=ht_ps[:, :nn],
                                func=mybir.ActivationFunctionType.Relu,
                                bias=b1_sb[:, h:h + 1], scale=1.0,
                            )
                # --- layer 2 transposed: y[o, n] = sum_h W2q^T @ ht ---
                y = yp.tile([128, 2 * 512], BF16, tag="y")
                if cfg.merged_y and not cfg.add_b2:
                    y_ps = ps_y.tile([128, 1024], f32, tag="yps")
                    for oh in range(2):
                        for hh in range(2):
                            nc.tensor.matmul(
                                out=y_ps[:, oh * 512: oh * 512 + nn],
                                lhsT=w2_sb[:, (hh * 2 + oh) * 128:
                                           (hh * 2 + oh + 1) * 128],
                                rhs=ht[:, hh * 512: hh * 512 + nn],
                                start=(hh == 0), stop=(hh == 1),
                            )
                    yv = y[:].rearrange("p (o n) -> p o n", o=2)[:, :, :nn]
                    ypv = y_ps[:].rearrange("p (o n) -> p o n",
                                            o=2)[:, :, :nn]
                    if gi % 2 == 0:
                        nc.scalar.activation(
                            out=yv, in_=ypv,
                            func=mybir.ActivationFunctionType.Copy)
                    else:
                        nc.vector.tensor_copy(out=yv, in_=ypv)
                else:
                    for oh in range(2):
                        y_ps = ps_y.tile([128, 512], f32, tag="yps")
                        for hh in range(2):
                            nc.tensor.matmul(
                                out=y_ps[:, :nn],
                                lhsT=w2_sb[:, (hh * 2 + oh) * 128:
                                           (hh * 2 + oh + 1) * 128],
                                rhs=ht[:, hh * 512: hh * 512 + nn],
                                start=(hh == 0), stop=(hh == 1),
                            )
                        # (oh ^ gi%2) alternation keeps DVE/ACT evenly fed
                        if cfg.add_b2:
                            nc.scalar.activation(
                                out=y[:, oh * 512: oh * 512 + nn],
                                in_=y_ps[:, :nn],
                                func=mybir.ActivationFunctionType.Identity,
                                bias=b2_sb[:, oh:oh + 1], scale=1.0,
                            )
                        elif (oh + gi) % 2 == 0:
                            nc.scalar.activation(
                                out=y[:, oh * 512: oh * 512 + nn],
                                in_=y_ps[:, :nn],
                                func=mybir.ActivationFunctionType.Copy,
                            )
                        else:
                            nc.vector.tensor_copy(
                                out=y[:, oh * 512: oh * 512 + nn],
                                in_=y_ps[:, :nn],
                            )
                if cfg.do_store:
                    n0 = nbase + tg0 * 128
                    eng0 = nc.sync if cfg.stores_split else nc.gpsimd
                    eng0.dma_start(out=out_d[0, :, n0:n0 + nn],
                                   in_=y[:, 0:nn])
                    nc.gpsimd.dma_start(out=out_d[1, :, n0:n0 + nn],
                                        in_=y[:, 512: 512 + nn])

            def body():
                gi = 0
                for b in range(B):
                    for ci, (t0, kt) in enumerate(cfg.chunk_plan_for(b)):
                        # --- dense load of host-gathered corners (bf16) ---
                        if cfg.do_load or cfg.do_mult:
                            g = gp.tile([128, cfg.sc * 256], BF16, tag="g")
                        if cfg.do_load:
                            nc.sync.dma_start(out=g[:, :kt * 256],
                                              in_=gc_d[b, ci, :, :kt * 256])
                        elif cfg.do_mult:
                            nc.gpsimd.memset(g[:, :kt * 256], 0.25)
                        # --- weighted corners: scaled = g * w ---
                        if cfg.do_mult or cfg.do_reduce:
                            scaled = spool.tile([128, cfg.sc * 256], BF16,
                                                tag="s")
                        if cfg.do_mult:
                            g_v = g[:, :kt * 256].rearrange(
                                "p (t k c) -> p t k c", k=4, c=64)
                            w_v = (
                                w_sb[:, t0 * 4:(t0 + kt) * 4]
                                .rearrange("p (t k o) -> p t k o", k=4, o=1)
                                .to_broadcast([128, kt, 4, 64])
                            )
                            s_v = scaled[:, :kt * 256].rearrange(
                                "p (t k c) -> p t k c", k=4, c=64)
                            mult_eng = (nc.vector if cfg.mult_on_dve
                                        else nc.gpsimd)
                            mult_eng.tensor_tensor(out=s_v, in0=g_v, in1=w_v,
                                                   op=mybir.AluOpType.mult)
                        elif cfg.do_reduce:
                            nc.gpsimd.memset(scaled[:, :kt * 256], 0.25)
                        # --- corner sum -> x [128, kt*64] (bf16) ---
                        pe_red = cfg.pe_reduce == 2 or (
                            cfg.pe_reduce == 1 and ci % 2 == 1)
                        if pe_red:
                            src = scaled
                        else:
                            x = xp.tile([128, cfg.sc * 64], BF16, tag="x")
                            src = x
                            if not cfg.do_reduce and cfg.do_mm:
                                nc.gpsimd.memset(x[:, :kt * 64], 0.25)
                            if cfg.do_reduce and cfg.pool_reduce:
                                s4 = scaled[:, :kt * 256].rearrange(
                                    "p (t k c) -> p t k c", k=4, c=64)
                                ra = rp.tile([128, cfg.sc * 64], BF16,
                                             tag="ra")
                                rb = rp.tile([128, cfg.sc * 64], BF16,
                                             tag="rb")
                                ra_v = ra[:, :kt * 64].rearrange(
                                    "p (t c) -> p t c", c=64)
                                rb_v = rb[:, :kt * 64].rearrange(
                                    "p (t c) -> p t c", c=64)
                                x_v = x[:, :kt * 64].rearrange(
                                    "p (t c) -> p t c", c=64)
                                nc.gpsimd.tensor_tensor(
                                    out=ra_v, in0=s4[:, :, 0, :],
                                    in1=s4[:, :, 1, :],
                                    op=mybir.AluOpType.add)
                                nc.gpsimd.tensor_tensor(
                                    out=rb_v, in0=s4[:, :, 2, :],
                                    in1=s4[:, :, 3, :],
                                    op=mybir.AluOpType.add)
                                nc.gpsimd.tensor_tensor(
                                    out=x_v, in0=ra_v, in1=rb_v,
                                    op=mybir.AluOpType.add)
                            elif cfg.do_reduce:
                                with nc.allow_low_precision(
                                        reason="4-term bf16 sum, tol 2e-2"):
                                    nc.vector.tensor_reduce(
                                        out=x[:, :kt * 64].rearrange(
                                            "p (t c) -> p t c", c=64),
                                        in_=scaled[:, :kt * 256].rearrange(
                                            "p (t k c) -> p t c k",
                                            k=4, c=64),
                                        axis=mybir.AxisListType.X,
                                        op=mybir.AluOpType.add,
                                    )
                        if cfg.do_mm:
                            for g0 in range(0, kt, 4):
                                gt = min(4, kt - g0)
                                mm_group(gi, src, g0, b * cfg.npc,
                                         t0 + g0, gt, pe_red)
                                gi += 1

            if cfg.loop_k > 0:
                with tc.For_i(0, cfg.loop_k, 1,
                              staggered_reset=cfg.staggered):
                    body()
            else:
                body()
    nc.compile()
    return nc


# ---------------------------------------------------------------------------
# Host side
# ---------------------------------------------------------------------------

_NC_CACHE = {}


def _get_nc(cfg: Cfg):
    key = (cfg.add_b2, cfg.npc, cfg.loop_k)
    if key not in _NC_CACHE:
        _NC_CACHE[key] = build_nc(cfg)
    return _NC_CACHE[key]


def _core_layout(arr, npc, core, width):
    """arr: (M_pad, width) -> per-core [128, tiles*width] partition-major."""
    t = npc // 128
    a = arr[core * npc:(core + 1) * npc]
    return np.ascontiguousarray(
        a.reshape(t, 128, width).transpose(1, 0, 2).reshape(128, t * width)
    )


def make_in_maps(grid_data, mesh_features, indices, weights, W1, b1, W2, b2,
                 cfg):
    grid_data = np.asarray(grid_data, dtype=np.float32)
    mesh_features = np.asarray(mesh_features, dtype=np.float32)
    indices = np.asarray(indices).astype(np.int64)
    weights = np.asarray(weights, dtype=np.float32)
    npc = cfg.npc
    m_pad = N_CORES * npc
    T = cfg.tiles

    grid2d = grid_data.reshape(B * G, C).astype(NPDT)

    wp = np.zeros((m_pad, 4), dtype=np.float32)
    wp[:M] = weights
    mfp = np.zeros((m_pad, F), dtype=np.float32)
    mfp[:M] = mesh_features
    idxp = np.zeros((m_pad, 4), dtype=np.int64)
    idxp[:M] = indices

    b1r = np.ascontiguousarray(np.asarray(b1, np.float32).reshape(2, 128).T)
    # W2 quadrants [hh*2+oh, 128, 128]
    w2 = np.asarray(W2, np.float32)
    w2q = np.stack([w2[hh * 128:(hh + 1) * 128, oh * 128:(oh + 1) * 128]
                    for hh in range(2) for oh in range(2)]).astype(NPDT)
    b2r = np.ascontiguousarray(
        np.asarray(b2, np.float32).reshape(2, 128).T)

    in_maps = []
    for c in range(N_CORES):
        idx_c = idxp[c * npc:(c + 1) * npc]  # (npc, 4)
        gcorn = np.zeros((B, cfg.n_chunks, 128, cfg.sc * 256), dtype=NPDT)
        for b in range(B):
            # (npc, 4, C) -> tiles (T,128,4,C) -> (128, T, 4*C)
            g4 = grid2d[b * G + idx_c]
            g4 = g4.reshape(T, 128, 4 * C).transpose(1, 0, 2)
            for ci, (t0, kt) in enumerate(cfg.chunk_plan_for(b)):
                gcorn[b, ci, :, :kt * 256] = (
                    g4[:, t0:t0 + kt].reshape(128, kt * 256))
        im = {
            "gcorn": gcorn,
            "wts": _core_layout(wp, npc, c, 4).astype(NPDT),
            "mf": _core_layout(mfp, npc, c, F).astype(NPDT),
            "W1": np.asarray(W1, np.float32).astype(NPDT),
            "b1r": b1r,
            "W2q": w2q,
        }
        if cfg.add_b2:
            im["b2r"] = b2r
        in_maps.append(im)
    return in_maps


def kernel(grid_data, mesh_features, indices, weights, W1, b1, W2, b2):
    cfg = Cfg(add_b2=bool(np.any(np.asarray(b2))),
              add_b1=bool(np.any(np.asarray(b1))))
    nc = _get_nc(cfg)
    in_maps = make_in_maps(grid_data, mesh_features, indices, weights,
                           W1, b1, W2, b2, cfg)

    from concourse.bass_utils import run_bass_kernel_spmd
    res = run_bass_kernel_spmd(nc, in_maps, core_ids=list(range(N_CORES)))

    npc = cfg.npc
    # per-core out: [2(outhalf), 128, B*npc] bf16, nodes ordered [b, node]
    shards = []
    for c in range(N_CORES):
        o = np.asarray(res.results[c]["out"])  # (2, 128, B*npc)
        o = o.reshape(2, 128, B, npc).transpose(2, 3, 0, 1)  # (B,npc,2,128)
        shards.append(o.reshape(B, npc, OUT))
    y = np.concatenate(shards, axis=1)[:, :M, :].astype(np.float32)
    return np.ascontiguousarray(y)


# revision 69
# speedup vs baseline: 1.0183x; 1.0080x over previous
"""Trainium2 Bass kernel for nn_GridToMeshEncoder.

Computes: bilinear 4-corner gather from a (B,721,1440,64) grid at 40962 mesh
nodes + weighted corner sum, concat 4 mesh features, 2-layer MLP (68->256->256).

Strategy (v5): mesh nodes sharded across 8 NeuronCores (5248 padded nodes per
core, both batches on every core). The irregular corner gather runs on the
host (TRN2 indirect DMA is descriptor-rate-limited for 256B rows — measured
4x slower than the dense-DMA floor), packed into the exact partition-major
bf16 tile layout the device consumes. The device pipeline is bf16 end-to-end
(PSUM accumulation in fp32), spread across all five engine queues so each
stays under ~50% busy: corner-weight multiply on Pool, 4-corner reduction on
DVE, per-tile PE transposes (x and mesh-feature tiles) feed a W1 matmul with
nodes streaming on the free dim at N=512, relu+bias on ACT, and the second
layer computed transposed (y_t = W2q^T @ h) so the four W2 quadrants are the
stationary operands. PSUM->SBUF drains alternate between ACT and DVE per
chunk; output stores alternate between the SP (HWDGE) and Pool (SWDGE) DMA
queues. The device emits y transposed in bf16; the host widens to fp32 and
un-transposes while assembling the full output (exact widening — the only
loss is bf16 rounding, rel err ~4.5e-3, well inside the 2e-2 gate).

Self-contained: hardcodes all shapes; imports bass from /opt/trn_rl_repo.
"""

import sys
from dataclasses import dataclass

import numpy as np

_TRN_REPO = "/opt/trn_rl_repo"
if _TRN_REPO not in sys.path:
    sys.path.insert(0, _TRN_REPO)

import concourse.mybir as mybir  # noqa: E402
import concourse.tile as tile  # noqa: E402
from concourse import bacc  # noqa: E402
from concourse.masks import make_identity  # noqa: E402

# Problem constants
B = 2
N_LAT, N_LON = 721, 1440
G = N_LAT * N_LON  # 1038240 grid rows per batch
C = 64  # grid channels
M = 40962  # mesh nodes
F = 4  # mesh features
DIN = C + F  # 68
HID = 256
OUT = 256
N_CORES = 8

BF16 = mybir.dt.bfloat16
NPDT = mybir.dt.np(BF16)


@dataclass(frozen=True)
class Cfg:
    npc: int = 5248  # nodes per core (41 tiles of 128)
    add_b2: bool = False
    add_b1: bool = False
    loop_k: int = 0  # >0: wrap compute in a hardware loop (timing builds)
    bufs: tuple = (4, 3, 3, 3, 3, 3)  # gp, spool, xp, xtp, htp, yp
    mult_on_dve: bool = False  # weighted-corner multiply on DVE vs Pool
    sc: int = 4  # superchunk size in tiles (load/mult/reduce granularity)
    taper: int = 0  # 0: none; 1: 1,2,4..4,2 chunk plan; 2: 2,4..4,3
    stores_split: bool = True  # out-DMAs alternate SP/Pool vs all on Pool
    swap_stores: bool = False  # oh0 -> Pool (SWDGE), oh1 -> SP (HWDGE)
    relu_split: bool = False  # relu h=1 on DVE (tensor_scalar) vs both ACT
    ps_bufs: tuple = (2, 3, 3)  # ps_xt, ps_ht, ps_y (<= 8 banks total)
    merged_y: bool = False  # y PSUM as one [128,1024] tile, single drain op
    staggered: bool = False  # For_i staggered_reset (no per-iter barrier)
    # 0: k-reduce on DVE always; 1: odd chunks reduce on PE via
    # accumulating transposes; 2: always on PE
    pe_reduce: int = 0
    merged_ht: bool = False  # ht PSUM as one [128,1024] tile, single relu op
    pool_reduce: bool = False  # k-reduce as 3 tensor adds on Pool
    # timing-ablation switches (correctness only valid when all True)
    do_load: bool = True
    do_mult: bool = True
    do_reduce: bool = True
    do_mm: bool = True
    do_store: bool = True

    @property
    def tiles(self):
        assert self.npc % 128 == 0
        return self.npc // 128

    def _plan_from_widths(self, widths):
        if widths is None or sum(widths) != self.tiles:
            widths = None
        plan, t = [], 0
        if widths is not None:
            for k in widths:
                plan.append((t, k))
                t += k
            return plan
        while t < self.tiles:
            k = min(self.sc, self.tiles - t)
            plan.append((t, k))
            t += k
        return plan

    def chunk_plan_for(self, b):
        # optional taper: narrow first (and last) chunks prime/drain the
        # cross-engine pipeline faster than full-width ones
        full = (self.tiles - 5) // 4
        if self.taper == 1 and self.sc == 4:
            widths = [1, 2] + [4] * full + [2]
        elif self.taper == 2 and self.sc == 4:
            widths = [2] + [4] * full + [3]
        elif self.taper == 3 and self.sc == 4:
            # taper only at body start (b=0) and body end (b=B-1)
            if b == 0:
                widths = [1, 2] + [4] * full + [2]
            else:
                widths = [2] + [4] * full + [2, 1]
        else:
            widths = None
        return self._plan_from_widths(widths)

    @property
    def chunk_plan(self):
        return self.chunk_plan_for(0)

    @property
    def n_chunks(self):
        return len(self.chunk_plan)


def build_nc(cfg: Cfg):
    """Build the per-core Bass program (identical across all 8 cores)."""
    f32 = mybir.dt.float32
    nc = bacc.Bacc("TRN2", target_bir_lowering=False, debug=False)
    T = cfg.tiles
    NCH = cfg.n_chunks

    # host-gathered corners, bf16: [b, chunk, p, t*256 + k*64 + c]
    gc_d = nc.dram_tensor("gcorn", [B, NCH, 128, cfg.sc * 256], BF16,
                          kind="ExternalInput")
    w_d = nc.dram_tensor("wts", [128, T * 4], BF16, kind="ExternalInput")
    mf_d = nc.dram_tensor("mf", [128, T * F], BF16, kind="ExternalInput")
    w1_d = nc.dram_tensor("W1", [DIN, HID], BF16, kind="ExternalInput")
    b1_d = nc.dram_tensor("b1r", [128, 2], f32, kind="ExternalInput")
    # W2 quadrants: [hidhalf*2+outhalf, 128 hid, 128 out]
    w2_d = nc.dram_tensor("W2q", [4, 128, 128], BF16, kind="ExternalInput")
    if cfg.add_b2:
        b2_d = nc.dram_tensor("b2r", [128, 2], f32, kind="ExternalInput")
    # output transposed: [outhalf, outch(128), b*npc + node]
    out_d = nc.dram_tensor("out", [2, 128, B * cfg.npc], BF16,
                           kind="ExternalOutput")

    with tile.TileContext(nc) as tc:
        with (
            tc.tile_pool(name="res", bufs=1) as res,
            tc.tile_pool(name="gp", bufs=cfg.bufs[0]) as gp,
            tc.tile_pool(name="sp", bufs=cfg.bufs[1]) as spool,
            tc.tile_pool(name="xp", bufs=cfg.bufs[2]) as xp,
            tc.tile_pool(name="rp", bufs=4) as rp,
            tc.tile_pool(name="xtp", bufs=cfg.bufs[3]) as xtp,
            tc.tile_pool(name="htp", bufs=cfg.bufs[4]) as htp,
            tc.tile_pool(name="yp", bufs=cfg.bufs[5]) as yp,
            tc.tile_pool(name="ps_xt", bufs=cfg.ps_bufs[0],
                         space="PSUM") as ps_xt,
            tc.tile_pool(name="ps_ht", bufs=cfg.ps_bufs[1],
                         space="PSUM") as ps_ht,
            tc.tile_pool(name="ps_y", bufs=cfg.ps_bufs[2],
                         space="PSUM") as ps_y,
        ):
            w_sb = res.tile([128, T * 4], BF16)
            mf_sb = res.tile([128, T * F], BF16)
            w1_sb = res.tile([DIN, HID], BF16)
            b1_sb = res.tile([128, 2], f32)
            w2_sb = res.tile([128, 4 * 128], BF16)
            ident = res.tile([128, 128], BF16)

            nc.sync.dma_start(out=w_sb[:], in_=w_d[:])
            nc.sync.dma_start(out=mf_sb[:], in_=mf_d[:])
            nc.sync.dma_start(out=w1_sb[:], in_=w1_d[:])
            nc.sync.dma_start(out=b1_sb[:], in_=b1_d[:])
            for q in range(4):
                nc.sync.dma_start(out=w2_sb[:, q * 128:(q + 1) * 128],
                                  in_=w2_d[q])
            if cfg.add_b2:
                b2_sb = res.tile([128, 2], f32)
                nc.sync.dma_start(out=b2_sb[:], in_=b2_d[:])
            make_identity(nc, ident[:])

            def mm_group(gi, src, xoff, nbase, tg0, gt, pe_red=False):
                """4-tile MM group: transpose -> W1 -> relu -> W2 -> store.

                pe_red: src is `scaled` [128, sc*256]; the 4-corner sum runs
                on the PE as 4 accumulating transposes per tile. Otherwise
                src is the DVE-reduced x [128, sc*64], one transpose per tile.
                """
                nn = gt * 128
                xt_ps = ps_xt.tile([DIN, 4 * 128], BF16, tag="xtps")
                for tl in range(gt):
                    if pe_red:
                        for k in range(4):
                            col = ((xoff + tl) * 4 + k) * 64
                            nc.tensor.matmul(
                                out=xt_ps[0:64, tl * 128:(tl + 1) * 128],
                                lhsT=src[:, col:col + 64],
                                rhs=ident[:],
                                is_transpose=True,
                                start=(k == 0), stop=(k == 3),
                            )
                    else:
                        nc.tensor.transpose(
                            out=xt_ps[0:64, tl * 128:(tl + 1) * 128],
                            in_=src[:, (xoff + tl) * 64:(xoff + tl + 1) * 64],
                            identity=ident[:],
                        )
                    nc.tensor.transpose(
                        out=xt_ps[64:68, tl * 128:(tl + 1) * 128],
                        in_=mf_sb[:, (tg0 + tl) * 4:(tg0 + tl + 1) * 4],
                        identity=ident[:],
                    )
                xt = xtp.tile([DIN, 4 * 128], BF16, tag="xt")
                if gi % 2 == 0 or cfg.relu_split:
                    nc.scalar.activation(
                        out=xt[:, :nn], in_=xt_ps[:, :nn],
                        func=mybir.ActivationFunctionType.Copy)
                else:
                    nc.vector.tensor_copy(out=xt[:, :nn], in_=xt_ps[:, :nn])
                # --- layer 1: ht[h, n] = W1h^T @ xt ---
                ht = htp.tile([128, 2 * 512], BF16, tag="ht")
                if cfg.merged_ht and not cfg.add_b1:
                    ht_ps = ps_ht.tile([128, 1024], f32, tag="htps")
                    for h in range(2):
                        nc.tensor.matmul(
                            out=ht_ps[:, h * 512: h * 512 + nn],
                            lhsT=w1_sb[:, h * 128:(h + 1) * 128],
                            rhs=xt[:, :nn], start=True, stop=True,
                        )
                    # single relu over both hid halves (b1 known zero)
                    nc.scalar.activation(
                        out=ht[:].rearrange("p (o n) -> p o n",
                                            o=2)[:, :, :nn],
                        in_=ht_ps[:].rearrange("p (o n) -> p o n",
                                               o=2)[:, :, :nn],
                        func=mybir.ActivationFunctionType.Relu,
                    )
                else:
                    for h in range(2):
                        ht_ps = ps_ht.tile([128, 512], f32, tag="htps")
                        nc.tensor.matmul(
                            out=ht_ps[:, :nn],
                            lhsT=w1_sb[:, h * 128:(h + 1) * 128],
                            rhs=xt[:, :nn], start=True, stop=True,
                        )
                        if cfg.relu_split and h == 1:
                            nc.vector.tensor_scalar(
                                out=ht[:, h * 512: h * 512 + nn],
                                in0=ht_ps[:, :nn],
                                scalar1=b1_sb[:, h:h + 1], scalar2=0.0,
                                op0=mybir.AluOpType.add,
                                op1=mybir.AluOpType.max,
                            )
                        else:
                            nc.scalar.activation(
                                out=ht[:, h * 512: h * 512 + nn],
                                in_=ht_ps[:, :nn],
                                func=mybir.ActivationFunctionType.Relu,
                                bias=b1_sb[:, h:h + 1], scale=1.0,
                            )
                # --- layer 2 transposed: y[o, n] = sum_h W2q^T @ ht ---
                y = yp.tile([128, 2 * 512], BF16, tag="y")
                if cfg.merged_y and not cfg.add_b2:
                    y_ps = ps_y.tile([128, 1024], f32, tag="yps")
                    for oh in range(2):
                        for hh in range(2):
                            nc.tensor.matmul(
                                out=y_ps[:, oh * 512: oh * 512 + nn],
                                lhsT=w2_sb[:, (hh * 2 + oh) * 128:
                                           (hh * 2 + oh + 1) * 128],
                                rhs=ht[:, hh * 512: hh * 512 + nn],
                                start=(hh == 0), stop=(hh == 1),
                            )
                    yv = y[:].rearrange("p (o n) -> p o n", o=2)[:, :, :nn]
                    ypv = y_ps[:].rearrange("p (o n) -> p o n",
                                            o=2)[:, :, :nn]
                    if gi % 2 == 0:
                        nc.scalar.activation(
                            out=yv, in_=ypv,
                            func=mybir.ActivationFunctionType.Copy)
                    else:
                        nc.vector.tensor_copy(out=yv, in_=ypv)
                else:
                    for oh in range(2):
                        y_ps = ps_y.tile([128, 512], f32, tag="yps")
                        for hh in range(2):
                            nc.tensor.matmul(
                                out=y_ps[:, :nn],
                                lhsT=w2_sb[:, (hh * 2 + oh) * 128:
                                           (hh * 2 + oh + 1) * 128],
                                rhs=ht[:, hh * 512: hh * 512 + nn],
                                start=(hh == 0), stop=(hh == 1),
                            )
                        # (oh ^ gi%2) alternation keeps DVE/ACT evenly fed
                        if cfg.add_b2:
                            nc.scalar.activation(
                                out=y[:, oh * 512: oh * 512 + nn],
                                in_=y_ps[:, :nn],
                                func=mybir.ActivationFunctionType.Identity,
                                bias=b2_sb[:, oh:oh + 1], scale=1.0,
                            )
                        elif (oh + gi) % 2 == 0:
                            nc.scalar.activation(
                                out=y[:, oh * 512: oh * 512 + nn],
                                in_=y_ps[:, :nn],
                                func=mybir.ActivationFunctionType.Copy,
                            )
                        else:
                            nc.vector.tensor_copy(
                                out=y[:, oh * 512: oh * 512 + nn],
                                in_=y_ps[:, :nn],
                            )
                if cfg.do_store:
                    n0 = nbase + tg0 * 128
                    eng0 = nc.sync if cfg.stores_split else nc.gpsimd
                    eng1 = nc.gpsimd
                    if cfg.swap_stores:
                        eng0, eng1 = eng1, eng0
                    eng0.dma_start(out=out_d[0, :, n0:n0 + nn],
                                   in_=y[:, 0:nn])
                    eng1.dma_start(out=out_d[1, :, n0:n0 + nn],
                                   in_=y[:, 512: 512 + nn])

            def body():
                gi = 0
                for b in range(B):
                    for ci, (t0, kt) in enumerate(cfg.chunk_plan_for(b)):
                        # --- dense load of host-gathered corners (bf16) ---
                        if cfg.do_load or cfg.do_mult:
                            g = gp.tile([128, cfg.sc * 256], BF16, tag="g")
                        if cfg.do_load:
                            nc.sync.dma_start(out=g[:, :kt * 256],
                                              in_=gc_d[b, ci, :, :kt * 256])
                        elif cfg.do_mult:
                            nc.gpsimd.memset(g[:, :kt * 256], 0.25)
                        # --- weighted corners: scaled = g * w ---
                        if cfg.do_mult or cfg.do_reduce:
                            scaled = spool.tile([128, cfg.sc * 256], BF16,
                                                tag="s")
                        if cfg.do_mult:
                            g_v = g[:, :kt * 256].rearrange(
                                "p (t k c) -> p t k c", k=4, c=64)
                            w_v = (
                                w_sb[:, t0 * 4:(t0 + kt) * 4]
                                .rearrange("p (t k o) -> p t k o", k=4, o=1)
                                .to_broadcast([128, kt, 4, 64])
                            )
                            s_v = scaled[:, :kt * 256].rearrange(
                                "p (t k c) -> p t k c", k=4, c=64)
                            mult_eng = (nc.vector if cfg.mult_on_dve
                                        else nc.gpsimd)
                            mult_eng.tensor_tensor(out=s_v, in0=g_v, in1=w_v,
                                                   op=mybir.AluOpType.mult)
                        elif cfg.do_reduce:
                            nc.gpsimd.memset(scaled[:, :kt * 256], 0.25)
                        # --- corner sum -> x [128, kt*64] (bf16) ---
                        pe_red = cfg.pe_reduce == 2 or (
                            cfg.pe_reduce == 1 and ci % 2 == 1)
                        if pe_red:
                            src = scaled
                        else:
                            x = xp.tile([128, cfg.sc * 64], BF16, tag="x")
                            src = x
                            if not cfg.do_reduce and cfg.do_mm:
                                nc.gpsimd.memset(x[:, :kt * 64], 0.25)
                            if cfg.do_reduce and cfg.pool_reduce:
                                s4 = scaled[:, :kt * 256].rearrange(
                                    "p (t k c) -> p t k c", k=4, c=64)
                                ra = rp.tile([128, cfg.sc * 64], BF16,
                                             tag="ra")
                                rb = rp.tile([128, cfg.sc * 64], BF16,
                                             tag="rb")
                                ra_v = ra[:, :kt * 64].rearrange(
                                    "p (t c) -> p t c", c=64)
                                rb_v = rb[:, :kt * 64].rearrange(
                                    "p (t c) -> p t c", c=64)
                                x_v = x[:, :kt * 64].rearrange(
                                    "p (t c) -> p t c", c=64)
                                nc.gpsimd.tensor_tensor(
                                    out=ra_v, in0=s4[:, :, 0, :],
                                    in1=s4[:, :, 1, :],
                                    op=mybir.AluOpType.add)
                                nc.gpsimd.tensor_tensor(
                                    out=rb_v, in0=s4[:, :, 2, :],
                                    in1=s4[:, :, 3, :],
                                    op=mybir.AluOpType.add)
                                nc.gpsimd.tensor_tensor(
                                    out=x_v, in0=ra_v, in1=rb_v,
                                    op=mybir.AluOpType.add)
                            elif cfg.do_reduce:
                                with nc.allow_low_precision(
                                        reason="4-term bf16 sum, tol 2e-2"):
                                    nc.vector.tensor_reduce(
                                        out=x[:, :kt * 64].rearrange(
                                            "p (t c) -> p t c", c=64),
                                        in_=scaled[:, :kt * 256].rearrange(
                                            "p (t k c) -> p t c k",
                                            k=4, c=64),
                                        axis=mybir.AxisListType.X,
                                        op=mybir.AluOpType.add,
                                    )
                        if cfg.do_mm:
                            for g0 in range(0, kt, 4):
                                gt = min(4, kt - g0)
                                mm_group(gi, src, g0, b * cfg.npc,
                                         t0 + g0, gt, pe_red)
                                gi += 1

            if cfg.loop_k > 0:
                with tc.For_i(0, cfg.loop_k, 1,
                              staggered_reset=cfg.staggered):
                    body()
            else:
                body()
    nc.compile()
    return nc


# ---------------------------------------------------------------------------
# Host side
# ---------------------------------------------------------------------------

_NC_CACHE = {}


def _get_nc(cfg: Cfg):
    key = (cfg.add_b2, cfg.npc, cfg.loop_k)
    if key not in _NC_CACHE:
        _NC_CACHE[key] = build_nc(cfg)
    return _NC_CACHE[key]


def _core_layout(arr, npc, core, width):
    """arr: (M_pad, width) -> per-core [128, tiles*width] partition-major."""
    t = npc // 128
    a = arr[core * npc:(core + 1) * npc]
    return np.ascontiguousarray(
        a.reshape(t, 128, width).transpose(1, 0, 2).reshape(128, t * width)
    )


def make_in_maps(grid_data, mesh_features, indices, weights, W1, b1, W2, b2,
                 cfg):
    grid_data = np.asarray(grid_data, dtype=np.float32)
    mesh_features = np.asarray(mesh_features, dtype=np.float32)
    indices = np.asarray(indices).astype(np.int64)
    weights = np.asarray(weights, dtype=np.float32)
    npc = cfg.npc
    m_pad = N_CORES * npc
    T = cfg.tiles

    grid2d = grid_data.reshape(B * G, C).astype(NPDT)

    wp = np.zeros((m_pad, 4), dtype=np.float32)
    wp[:M] = weights
    mfp = np.zeros((m_pad, F), dtype=np.float32)
    mfp[:M] = mesh_features
    idxp = np.zeros((m_pad, 4), dtype=np.int64)
    idxp[:M] = indices

    b1r = np.ascontiguousarray(np.asarray(b1, np.float32).reshape(2, 128).T)
    # W2 quadrants [hh*2+oh, 128, 128]
    w2 = np.asarray(W2, np.float32)
    w2q = np.stack([w2[hh * 128:(hh + 1) * 128, oh * 128:(oh + 1) * 128]
                    for hh in range(2) for oh in range(2)]).astype(NPDT)
    b2r = np.ascontiguousarray(
        np.asarray(b2, np.float32).reshape(2, 128).T)

    in_maps = []
    for c in range(N_CORES):
        idx_c = idxp[c * npc:(c + 1) * npc]  # (npc, 4)
        gcorn = np.zeros((B, cfg.n_chunks, 128, cfg.sc * 256), dtype=NPDT)
        for b in range(B):
            # (npc, 4, C) -> tiles (T,128,4,C) -> (128, T, 4*C)
            g4 = grid2d[b * G + idx_c]
            g4 = g4.reshape(T, 128, 4 * C).transpose(1, 0, 2)
            for ci, (t0, kt) in enumerate(cfg.chunk_plan_for(b)):
                gcorn[b, ci, :, :kt * 256] = (
                    g4[:, t0:t0 + kt].reshape(128, kt * 256))
        im = {
            "gcorn": gcorn,
            "wts": _core_layout(wp, npc, c, 4).astype(NPDT),
            "mf": _core_layout(mfp, npc, c, F).astype(NPDT),
            "W1": np.asarray(W1, np.float32).astype(NPDT),
            "b1r": b1r,
            "W2q": w2q,
        }
        if cfg.add_b2:
            im["b2r"] = b2r
        in_maps.append(im)
    return in_maps


def kernel(grid_data, mesh_features, indices, weights, W1, b1, W2, b2):
    cfg = Cfg(add_b2=bool(np.any(np.asarray(b2))),
              add_b1=bool(np.any(np.asarray(b1))))
    nc = _get_nc(cfg)
    in_maps = make_in_maps(grid_data, mesh_features, indices, weights,
                           W1, b1, W2, b2, cfg)

    from concourse.bass_utils import run_bass_kernel_spmd
    res = run_bass_kernel_spmd(nc, in_maps, core_ids=list(range(N_CORES)))

    npc = cfg.npc
    # per-core out: [2(outhalf), 128, B*npc] bf16, nodes ordered [b, node]
    shards = []
    for c in range(N_CORES):
        o = np.asarray(res.results[c]["out"])  # (2, 128, B*npc)
        o = o.reshape(2, 128, B, npc).transpose(2, 3, 0, 1)  # (B,npc,2,128)
        shards.append(o.reshape(B, npc, OUT))
    y = np.concatenate(shards, axis=1)[:, :M, :].astype(np.float32)
    return np.ascontiguousarray(y)


# revision 71
# speedup vs baseline: 1.0187x; 1.0004x over previous
"""Trainium2 Bass kernel for nn_GridToMeshEncoder.

Computes: bilinear 4-corner gather from a (B,721,1440,64) grid at 40962 mesh
nodes + weighted corner sum, concat 4 mesh features, 2-layer MLP (68->256->256).

Strategy (v5): mesh nodes sharded across 8 NeuronCores (5248 padded nodes per
core, both batches on every core). The irregular corner gather runs on the
host (TRN2 indirect DMA is descriptor-rate-limited for 256B rows — measured
4x slower than the dense-DMA floor), packed into the exact partition-major
bf16 tile layout the device consumes. The device pipeline is bf16 end-to-end
(PSUM accumulation in fp32), spread across all five engine queues so each
stays under ~50% busy: corner-weight multiply on Pool, 4-corner reduction on
DVE, per-tile PE transposes (x and mesh-feature tiles) feed a W1 matmul with
nodes streaming on the free dim at N=512, relu+bias on ACT, and the second
layer computed transposed (y_t = W2q^T @ h) so the four W2 quadrants are the
stationary operands. PSUM->SBUF drains alternate between ACT and DVE per
chunk; output stores alternate between the SP (HWDGE) and Pool (SWDGE) DMA
queues. The device emits y transposed in bf16; the host widens to fp32 and
un-transposes while assembling the full output (exact widening — the only
loss is bf16 rounding, rel err ~4.5e-3, well inside the 2e-2 gate).

Self-contained: hardcodes all shapes; imports bass from /opt/trn_rl_repo.
"""

import sys
from dataclasses import dataclass

import numpy as np

_TRN_REPO = "/opt/trn_rl_repo"
if _TRN_REPO not in sys.path:
    sys.path.insert(0, _TRN_REPO)

import concourse.mybir as mybir  # noqa: E402
import concourse.tile as tile  # noqa: E402
from concourse import bacc  # noqa: E402
from concourse.masks import make_identity  # noqa: E402

# Problem constants
B = 2
N_LAT, N_LON = 721, 1440
G = N_LAT * N_LON  # 1038240 grid rows per batch
C = 64  # grid channels
M = 40962  # mesh nodes
F = 4  # mesh features
DIN = C + F  # 68
HID = 256
OUT = 256
N_CORES = 8

BF16 = mybir.dt.bfloat16
NPDT = mybir.dt.np(BF16)


@dataclass(frozen=True)
class Cfg:
    npc: int = 5248  # nodes per core (41 tiles of 128)
    add_b2: bool = False
    add_b1: bool = False
    loop_k: int = 0  # >0: wrap compute in a hardware loop (timing builds)
    bufs: tuple = (4, 3, 3, 3, 3, 3)  # gp, spool, xp, xtp, htp, yp
    mult_on_dve: bool = False  # weighted-corner multiply on DVE vs Pool
    sc: int = 4  # superchunk size in tiles (load/mult/reduce granularity)
    taper: int = 0  # 0: none; 1: 1,2,4..4,2 chunk plan; 2: 2,4..4,3
    interleave_b: bool = False  # alternate batches per chunk (2 indep chains)
    stores_split: bool = True  # out-DMAs alternate SP/Pool vs all on Pool
    swap_stores: bool = False  # oh0 -> Pool (SWDGE), oh1 -> SP (HWDGE)
    relu_split: bool = False  # relu h=1 on DVE (tensor_scalar) vs both ACT
    ps_bufs: tuple = (2, 3, 3)  # ps_xt, ps_ht, ps_y (<= 8 banks total)
    merged_y: bool = False  # y PSUM as one [128,1024] tile, single drain op
    staggered: bool = False  # For_i staggered_reset (no per-iter barrier)
    # 0: k-reduce on DVE always; 1: odd chunks reduce on PE via
    # accumulating transposes; 2: always on PE
    pe_reduce: int = 0
    merged_ht: bool = False  # ht PSUM as one [128,1024] tile, single relu op
    pool_reduce: bool = False  # k-reduce as 3 tensor adds on Pool
    # timing-ablation switches (correctness only valid when all True)
    do_load: bool = True
    do_mult: bool = True
    do_reduce: bool = True
    do_mm: bool = True
    do_store: bool = True

    @property
    def tiles(self):
        assert self.npc % 128 == 0
        return self.npc // 128

    def _plan_from_widths(self, widths):
        if widths is None or sum(widths) != self.tiles:
            widths = None
        plan, t = [], 0
        if widths is not None:
            for k in widths:
                plan.append((t, k))
                t += k
            return plan
        while t < self.tiles:
            k = min(self.sc, self.tiles - t)
            plan.append((t, k))
            t += k
        return plan

    def chunk_plan_for(self, b):
        # optional taper: narrow first (and last) chunks prime/drain the
        # cross-engine pipeline faster than full-width ones
        full = (self.tiles - 5) // 4
        if self.taper == 1 and self.sc == 4:
            widths = [1, 2] + [4] * full + [2]
        elif self.taper == 2 and self.sc == 4:
            widths = [2] + [4] * full + [3]
        elif self.taper == 3 and self.sc == 4:
            # taper only at body start (b=0) and body end (b=B-1)
            if b == 0:
                widths = [1, 2] + [4] * full + [2]
            else:
                widths = [2] + [4] * full + [2, 1]
        else:
            widths = None
        return self._plan_from_widths(widths)

    @property
    def chunk_plan(self):
        return self.chunk_plan_for(0)

    @property
    def n_chunks(self):
        return len(self.chunk_plan)


def build_nc(cfg: Cfg):
    """Build the per-core Bass program (identical across all 8 cores)."""
    f32 = mybir.dt.float32
    nc = bacc.Bacc("TRN2", target_bir_lowering=False, debug=False)
    T = cfg.tiles
    NCH = cfg.n_chunks

    # host-gathered corners, bf16: [b, chunk, p, t*256 + k*64 + c]
    gc_d = nc.dram_tensor("gcorn", [B, NCH, 128, cfg.sc * 256], BF16,
                          kind="ExternalInput")
    w_d = nc.dram_tensor("wts", [128, T * 4], BF16, kind="ExternalInput")
    mf_d = nc.dram_tensor("mf", [128, T * F], BF16, kind="ExternalInput")
    w1_d = nc.dram_tensor("W1", [DIN, HID], BF16, kind="ExternalInput")
    b1_d = nc.dram_tensor("b1r", [128, 2], f32, kind="ExternalInput")
    # W2 quadrants: [hidhalf*2+outhalf, 128 hid, 128 out]
    w2_d = nc.dram_tensor("W2q", [4, 128, 128], BF16, kind="ExternalInput")
    if cfg.add_b2:
        b2_d = nc.dram_tensor("b2r", [128, 2], f32, kind="ExternalInput")
    # output transposed: [outhalf, outch(128), b*npc + node]
    out_d = nc.dram_tensor("out", [2, 128, B * cfg.npc], BF16,
                           kind="ExternalOutput")

    with tile.TileContext(nc) as tc:
        with (
            tc.tile_pool(name="res", bufs=1) as res,
            tc.tile_pool(name="gp", bufs=cfg.bufs[0]) as gp,
            tc.tile_pool(name="sp", bufs=cfg.bufs[1]) as spool,
            tc.tile_pool(name="xp", bufs=cfg.bufs[2]) as xp,
            tc.tile_pool(name="rp", bufs=4) as rp,
            tc.tile_pool(name="xtp", bufs=cfg.bufs[3]) as xtp,
            tc.tile_pool(name="htp", bufs=cfg.bufs[4]) as htp,
            tc.tile_pool(name="yp", bufs=cfg.bufs[5]) as yp,
            tc.tile_pool(name="ps_xt", bufs=cfg.ps_bufs[0],
                         space="PSUM") as ps_xt,
            tc.tile_pool(name="ps_ht", bufs=cfg.ps_bufs[1],
                         space="PSUM") as ps_ht,
            tc.tile_pool(name="ps_y", bufs=cfg.ps_bufs[2],
                         space="PSUM") as ps_y,
        ):
            w_sb = res.tile([128, T * 4], BF16)
            mf_sb = res.tile([128, T * F], BF16)
            w1_sb = res.tile([DIN, HID], BF16)
            b1_sb = res.tile([128, 2], f32)
            w2_sb = res.tile([128, 4 * 128], BF16)
            ident = res.tile([128, 128], BF16)

            nc.sync.dma_start(out=w_sb[:], in_=w_d[:])
            nc.sync.dma_start(out=mf_sb[:], in_=mf_d[:])
            nc.sync.dma_start(out=w1_sb[:], in_=w1_d[:])
            nc.sync.dma_start(out=b1_sb[:], in_=b1_d[:])
            for q in range(4):
                nc.sync.dma_start(out=w2_sb[:, q * 128:(q + 1) * 128],
                                  in_=w2_d[q])
            if cfg.add_b2:
                b2_sb = res.tile([128, 2], f32)
                nc.sync.dma_start(out=b2_sb[:], in_=b2_d[:])
            make_identity(nc, ident[:])

            def mm_group(gi, src, xoff, nbase, tg0, gt, pe_red=False):
                """4-tile MM group: transpose -> W1 -> relu -> W2 -> store.

                pe_red: src is `scaled` [128, sc*256]; the 4-corner sum runs
                on the PE as 4 accumulating transposes per tile. Otherwise
                src is the DVE-reduced x [128, sc*64], one transpose per tile.
                """
                nn = gt * 128
                xt_ps = ps_xt.tile([DIN, 4 * 128], BF16, tag="xtps")
                for tl in range(gt):
                    if pe_red:
                        for k in range(4):
                            col = ((xoff + tl) * 4 + k) * 64
                            nc.tensor.matmul(
                                out=xt_ps[0:64, tl * 128:(tl + 1) * 128],
                                lhsT=src[:, col:col + 64],
                                rhs=ident[:],
                                is_transpose=True,
                                start=(k == 0), stop=(k == 3),
                            )
                    else:
                        nc.tensor.transpose(
                            out=xt_ps[0:64, tl * 128:(tl + 1) * 128],
                            in_=src[:, (xoff + tl) * 64:(xoff + tl + 1) * 64],
                            identity=ident[:],
                        )
                    nc.tensor.transpose(
                        out=xt_ps[64:68, tl * 128:(tl + 1) * 128],
                        in_=mf_sb[:, (tg0 + tl) * 4:(tg0 + tl + 1) * 4],
                        identity=ident[:],
                    )
                xt = xtp.tile([DIN, 4 * 128], BF16, tag="xt")
                if gi % 2 == 0 or cfg.relu_split:
                    nc.scalar.activation(
                        out=xt[:, :nn], in_=xt_ps[:, :nn],
                        func=mybir.ActivationFunctionType.Copy)
                else:
                    nc.vector.tensor_copy(out=xt[:, :nn], in_=xt_ps[:, :nn])
                # --- layer 1: ht[h, n] = W1h^T @ xt ---
                ht = htp.tile([128, 2 * 512], BF16, tag="ht")
                if cfg.merged_ht and not cfg.add_b1:
                    ht_ps = ps_ht.tile([128, 1024], f32, tag="htps")
                    for h in range(2):
                        nc.tensor.matmul(
                            out=ht_ps[:, h * 512: h * 512 + nn],
                            lhsT=w1_sb[:, h * 128:(h + 1) * 128],
                            rhs=xt[:, :nn], start=True, stop=True,
                        )
                    # single relu over both hid halves (b1 known zero)
                    nc.scalar.activation(
                        out=ht[:].rearrange("p (o n) -> p o n",
                                            o=2)[:, :, :nn],
                        in_=ht_ps[:].rearrange("p (o n) -> p o n",
                                               o=2)[:, :, :nn],
                        func=mybir.ActivationFunctionType.Relu,
                    )
                else:
                    for h in range(2):
                        ht_ps = ps_ht.tile([128, 512], f32, tag="htps")
                        nc.tensor.matmul(
                            out=ht_ps[:, :nn],
                            lhsT=w1_sb[:, h * 128:(h + 1) * 128],
                            rhs=xt[:, :nn], start=True, stop=True,
                        )
                        if cfg.relu_split and h == 1:
                            nc.vector.tensor_scalar(
                                out=ht[:, h * 512: h * 512 + nn],
                                in0=ht_ps[:, :nn],
                                scalar1=b1_sb[:, h:h + 1], scalar2=0.0,
                                op0=mybir.AluOpType.add,
                                op1=mybir.AluOpType.max,
                            )
                        else:
                            nc.scalar.activation(
                                out=ht[:, h * 512: h * 512 + nn],
                                in_=ht_ps[:, :nn],
                                func=mybir.ActivationFunctionType.Relu,
                                bias=b1_sb[:, h:h + 1], scale=1.0,
                            )
                # --- layer 2 transposed: y[o, n] = sum_h W2q^T @ ht ---
                y = yp.tile([128, 2 * 512], BF16, tag="y")
                if cfg.merged_y and not cfg.add_b2:
                    y_ps = ps_y.tile([128, 1024], f32, tag="yps")
                    for oh in range(2):
                        for hh in range(2):
                            nc.tensor.matmul(
                                out=y_ps[:, oh * 512: oh * 512 + nn],
                                lhsT=w2_sb[:, (hh * 2 + oh) * 128:
                                           (hh * 2 + oh + 1) * 128],
                                rhs=ht[:, hh * 512: hh * 512 + nn],
                                start=(hh == 0), stop=(hh == 1),
                            )
                    yv = y[:].rearrange("p (o n) -> p o n", o=2)[:, :, :nn]
                    ypv = y_ps[:].rearrange("p (o n) -> p o n",
                                            o=2)[:, :, :nn]
                    if gi % 2 == 0:
                        nc.scalar.activation(
                            out=yv, in_=ypv,
                            func=mybir.ActivationFunctionType.Copy)
                    else:
                        nc.vector.tensor_copy(out=yv, in_=ypv)
                else:
                    for oh in range(2):
                        y_ps = ps_y.tile([128, 512], f32, tag="yps")
                        for hh in range(2):
                            nc.tensor.matmul(
                                out=y_ps[:, :nn],
                                lhsT=w2_sb[:, (hh * 2 + oh) * 128:
                                           (hh * 2 + oh + 1) * 128],
                                rhs=ht[:, hh * 512: hh * 512 + nn],
                                start=(hh == 0), stop=(hh == 1),
                            )
                        # (oh ^ gi%2) alternation keeps DVE/ACT evenly fed
                        if cfg.add_b2:
                            nc.scalar.activation(
                                out=y[:, oh * 512: oh * 512 + nn],
                                in_=y_ps[:, :nn],
                                func=mybir.ActivationFunctionType.Identity,
                                bias=b2_sb[:, oh:oh + 1], scale=1.0,
                            )
                        elif (oh + gi) % 2 == 0:
                            nc.scalar.activation(
                                out=y[:, oh * 512: oh * 512 + nn],
                                in_=y_ps[:, :nn],
                                func=mybir.ActivationFunctionType.Copy,
                            )
                        else:
                            nc.vector.tensor_copy(
                                out=y[:, oh * 512: oh * 512 + nn],
                                in_=y_ps[:, :nn],
                            )
                if cfg.do_store:
                    n0 = nbase + tg0 * 128
                    eng0 = nc.sync if cfg.stores_split else nc.gpsimd
                    eng1 = nc.gpsimd
                    if cfg.swap_stores:
                        eng0, eng1 = eng1, eng0
                    eng0.dma_start(out=out_d[0, :, n0:n0 + nn],
                                   in_=y[:, 0:nn])
                    eng1.dma_start(out=out_d[1, :, n0:n0 + nn],
                                   in_=y[:, 512: 512 + nn])

            def body():
                gi = 0
                if cfg.interleave_b:
                    order = [(b, ci, t0, kt)
                             for ci, (t0, kt) in enumerate(cfg.chunk_plan)
                             for b in range(B)]
                else:
                    order = [(b, ci, t0, kt)
                             for b in range(B)
                             for ci, (t0, kt) in
                             enumerate(cfg.chunk_plan_for(b))]
                for b, ci, t0, kt in order:
                    if True:
                        # --- dense load of host-gathered corners (bf16) ---
                        if cfg.do_load or cfg.do_mult:
                            g = gp.tile([128, cfg.sc * 256], BF16, tag="g")
                        if cfg.do_load:
                            nc.sync.dma_start(out=g[:, :kt * 256],
                                              in_=gc_d[b, ci, :, :kt * 256])
                        elif cfg.do_mult:
                            nc.gpsimd.memset(g[:, :kt * 256], 0.25)
                        # --- weighted corners: scaled = g * w ---
                        if cfg.do_mult or cfg.do_reduce:
                            scaled = spool.tile([128, cfg.sc * 256], BF16,
                                                tag="s")
                        if cfg.do_mult:
                            g_v = g[:, :kt * 256].rearrange(
                                "p (t k c) -> p t k c", k=4, c=64)
                            w_v = (
                                w_sb[:, t0 * 4:(t0 + kt) * 4]
                                .rearrange("p (t k o) -> p t k o", k=4, o=1)
                                .to_broadcast([128, kt, 4, 64])
                            )
                            s_v = scaled[:, :kt * 256].rearrange(
                                "p (t k c) -> p t k c", k=4, c=64)
                            mult_eng = (nc.vector if cfg.mult_on_dve
                                        else nc.gpsimd)
                            mult_eng.tensor_tensor(out=s_v, in0=g_v, in1=w_v,
                                                   op=mybir.AluOpType.mult)
                        elif cfg.do_reduce:
                            nc.gpsimd.memset(scaled[:, :kt * 256], 0.25)
                        # --- corner sum -> x [128, kt*64] (bf16) ---
                        pe_red = cfg.pe_reduce == 2 or (
                            cfg.pe_reduce == 1 and ci % 2 == 1)
                        if pe_red:
                            src = scaled
                        else:
                            x = xp.tile([128, cfg.sc * 64], BF16, tag="x")
                            src = x
                            if not cfg.do_reduce and cfg.do_mm:
                                nc.gpsimd.memset(x[:, :kt * 64], 0.25)
                            if cfg.do_reduce and cfg.pool_reduce:
                                s4 = scaled[:, :kt * 256].rearrange(
                                    "p (t k c) -> p t k c", k=4, c=64)
                                ra = rp.tile([128, cfg.sc * 64], BF16,
                                             tag="ra")
                                rb = rp.tile([128, cfg.sc * 64], BF16,
                                             tag="rb")
                                ra_v = ra[:, :kt * 64].rearrange(
                                    "p (t c) -> p t c", c=64)
                                rb_v = rb[:, :kt * 64].rearrange(
                                    "p (t c) -> p t c", c=64)
                                x_v = x[:, :kt * 64].rearrange(
                                    "p (t c) -> p t c", c=64)
                                nc.gpsimd.tensor_tensor(
                                    out=ra_v, in0=s4[:, :, 0, :],
                                    in1=s4[:, :, 1, :],
                                    op=mybir.AluOpType.add)
                                nc.gpsimd.tensor_tensor(
                                    out=rb_v, in0=s4[:, :, 2, :],
                                    in1=s4[:, :, 3, :],
                                    op=mybir.AluOpType.add)
                                nc.gpsimd.tensor_tensor(
                                    out=x_v, in0=ra_v, in1=rb_v,
                                    op=mybir.AluOpType.add)
                            elif cfg.do_reduce:
                                with nc.allow_low_precision(
                                        reason="4-term bf16 sum, tol 2e-2"):
                                    nc.vector.tensor_reduce(
                                        out=x[:, :kt * 64].rearrange(
                                            "p (t c) -> p t c", c=64),
                                        in_=scaled[:, :kt * 256].rearrange(
                                            "p (t k c) -> p t c k",
                                            k=4, c=64),
                                        axis=mybir.AxisListType.X,
                                        op=mybir.AluOpType.add,
                                    )
                        if cfg.do_mm:
                            for g0 in range(0, kt, 4):
                                gt = min(4, kt - g0)
                                mm_group(gi, src, g0, b * cfg.npc,
                                         t0 + g0, gt, pe_red)
                                gi += 1

            if cfg.loop_k > 0:
                with tc.For_i(0, cfg.loop_k, 1,
                              staggered_reset=cfg.staggered):
                    body()
            else:
                body()
    nc.compile()
    return nc


# ---------------------------------------------------------------------------
# Host side
# ---------------------------------------------------------------------------

_NC_CACHE = {}


def _get_nc(cfg: Cfg):
    key = (cfg.add_b2, cfg.npc, cfg.loop_k)
    if key not in _NC_CACHE:
        _NC_CACHE[key] = build_nc(cfg)
    return _NC_CACHE[key]


def _core_layout(arr, npc, core, width):
    """arr: (M_pad, width) -> per-core [128, tiles*width] partition-major."""
    t = npc // 128
    a = arr[core * npc:(core + 1) * npc]
    return np.ascontiguousarray(
        a.reshape(t, 128, width).transpose(1, 0, 2).reshape(128, t * width)
    )


def make_in_maps(grid_data, mesh_features, indices, weights, W1, b1, W2, b2,
                 cfg):
    grid_data = np.asarray(grid_data, dtype=np.float32)
    mesh_features = np.asarray(mesh_features, dtype=np.float32)
    indices = np.asarray(indices).astype(np.int64)
    weights = np.asarray(weights, dtype=np.float32)
    npc = cfg.npc
    m_pad = N_CORES * npc
    T = cfg.tiles

    grid2d = grid_data.reshape(B * G, C).astype(NPDT)

    wp = np.zeros((m_pad, 4), dtype=np.float32)
    wp[:M] = weights
    mfp = np.zeros((m_pad, F), dtype=np.float32)
    mfp[:M] = mesh_features
    idxp = np.zeros((m_pad, 4), dtype=np.int64)
    idxp[:M] = indices

    b1r = np.ascontiguousarray(np.asarray(b1, np.float32).reshape(2, 128).T)
    # W2 quadrants [hh*2+oh, 128, 128]
    w2 = np.asarray(W2, np.float32)
    w2q = np.stack([w2[hh * 128:(hh + 1) * 128, oh * 128:(oh + 1) * 128]
                    for hh in range(2) for oh in range(2)]).astype(NPDT)
    b2r = np.ascontiguousarray(
        np.asarray(b2, np.float32).reshape(2, 128).T)

    in_maps = []
    for c in range(N_CORES):
        idx_c = idxp[c * npc:(c + 1) * npc]  # (npc, 4)
        gcorn = np.zeros((B, cfg.n_chunks, 128, cfg.sc * 256), dtype=NPDT)
        for b in range(B):
            # (npc, 4, C) -> tiles (T,128,4,C) -> (128, T, 4*C)
            g4 = grid2d[b * G + idx_c]
            g4 = g4.reshape(T, 128, 4 * C).transpose(1, 0, 2)
            for ci, (t0, kt) in enumerate(cfg.chunk_plan_for(b)):
                gcorn[b, ci, :, :kt * 256] = (
                    g4[:, t0:t0 + kt].reshape(128, kt * 256))
        im = {
            "gcorn": gcorn,
            "wts": _core_layout(wp, npc, c, 4).astype(NPDT),
            "mf": _core_layout(mfp, npc, c, F).astype(NPDT),
            "W1": np.asarray(W1, np.float32).astype(NPDT),
            "b1r": b1r,
            "W2q": w2q,
        }
        if cfg.add_b2:
            im["b2r"] = b2r
        in_maps.append(im)
    return in_maps


def kernel(grid_data, mesh_features, indices, weights, W1, b1, W2, b2):
    cfg = Cfg(add_b2=bool(np.any(np.asarray(b2))),
              add_b1=bool(np.any(np.asarray(b1))))
    nc = _get_nc(cfg)
    in_maps = make_in_maps(grid_data, mesh_features, indices, weights,
                           W1, b1, W2, b2, cfg)

    from concourse.bass_utils import run_bass_kernel_spmd
    res = run_bass_kernel_spmd(nc, in_maps, core_ids=list(range(N_CORES)))

    npc = cfg.npc
    # per-core out: [2(outhalf), 128, B*npc] bf16, nodes ordered [b, node]
    shards = []
    for c in range(N_CORES):
        o = np.asarray(res.results[c]["out"])  # (2, 128, B*npc)
        o = o.reshape(2, 128, B, npc).transpose(2, 3, 0, 1)  # (B,npc,2,128)
        shards.append(o.reshape(B, npc, OUT))
    y = np.concatenate(shards, axis=1)[:, :M, :].astype(np.float32)
    return np.ascontiguousarray(y)


# revision 76
# speedup vs baseline: 1.0562x; 1.0369x over previous
"""Trainium2 Bass kernel for nn_GridToMeshEncoder.

Computes: bilinear 4-corner gather from a (B,721,1440,64) grid at 40962 mesh
nodes + weighted corner sum, concat 4 mesh features, 2-layer MLP (68->256->256).

Strategy (v5): mesh nodes sharded across 8 NeuronCores (5248 padded nodes per
core, both batches on every core). The irregular corner gather runs on the
host (TRN2 indirect DMA is descriptor-rate-limited for 256B rows — measured
4x slower than the dense-DMA floor), packed into the exact partition-major
bf16 tile layout the device consumes. The device pipeline is bf16 end-to-end
(PSUM accumulation in fp32), spread across all five engine queues so each
stays under ~50% busy: corner-weight multiply on Pool, 4-corner reduction on
DVE, per-tile PE transposes (x and mesh-feature tiles) feed a W1 matmul with
nodes streaming on the free dim at N=512, relu+bias on ACT, and the second
layer computed transposed (y_t = W2q^T @ h) so the four W2 quadrants are the
stationary operands. PSUM->SBUF drains alternate between ACT and DVE per
chunk; output stores alternate between the SP (HWDGE) and Pool (SWDGE) DMA
queues. The device emits y transposed in bf16; the host widens to fp32 and
un-transposes while assembling the full output (exact widening — the only
loss is bf16 rounding, rel err ~4.5e-3, well inside the 2e-2 gate).

Self-contained: hardcodes all shapes; imports bass from /opt/trn_rl_repo.
"""

import sys
from dataclasses import dataclass

import numpy as np

_TRN_REPO = "/opt/trn_rl_repo"
if _TRN_REPO not in sys.path:
    sys.path.insert(0, _TRN_REPO)

import concourse.mybir as mybir  # noqa: E402
import concourse.tile as tile  # noqa: E402
from concourse import bacc  # noqa: E402
from concourse.masks import make_identity  # noqa: E402

# Problem constants
B = 2
N_LAT, N_LON = 721, 1440
G = N_LAT * N_LON  # 1038240 grid rows per batch
C = 64  # grid channels
M = 40962  # mesh nodes
F = 4  # mesh features
DIN = C + F  # 68
HID = 256
OUT = 256
N_CORES = 8

BF16 = mybir.dt.bfloat16
NPDT = mybir.dt.np(BF16)


@dataclass(frozen=True)
class Cfg:
    npc: int = 5248  # nodes per core (41 tiles of 128)
    add_b2: bool = False
    add_b1: bool = False
    loop_k: int = 0  # >0: wrap compute in a hardware loop (timing builds)
    bufs: tuple = (4, 3, 3, 3, 3, 3)  # gp, spool, xp, xtp, htp, yp
    mult_on_dve: bool = False  # weighted-corner multiply on DVE vs Pool
    sc: int = 4  # superchunk size in tiles (load/mult/reduce granularity)
    taper: int = 0  # 0: none; 1: 1,2,4..4,2 chunk plan; 2: 2,4..4,3
    interleave_b: bool = False  # alternate batches per chunk (2 indep chains)
    tck_layout: bool = True  # gcorn packed (t,c,k): k innermost, stride-1
    stores_split: bool = True  # out-DMAs alternate SP/Pool vs all on Pool
    swap_stores: bool = False  # oh0 -> Pool (SWDGE), oh1 -> SP (HWDGE)
    relu_split: bool = False  # relu h=1 on DVE (tensor_scalar) vs both ACT
    ps_bufs: tuple = (2, 3, 3)  # ps_xt, ps_ht, ps_y (<= 8 banks total)
    merged_y: bool = False  # y PSUM as one [128,1024] tile, single drain op
    staggered: bool = False  # For_i staggered_reset (no per-iter barrier)
    # 0: k-reduce on DVE always; 1: odd chunks reduce on PE via
    # accumulating transposes; 2: always on PE
    pe_reduce: int = 0
    merged_ht: bool = False  # ht PSUM as one [128,1024] tile, single relu op
    pool_reduce: bool = False  # k-reduce as 3 tensor adds on Pool
    # timing-ablation switches (correctness only valid when all True)
    do_load: bool = True
    do_mult: bool = True
    do_reduce: bool = True
    do_mm: bool = True
    do_store: bool = True

    @property
    def tiles(self):
        assert self.npc % 128 == 0
        return self.npc // 128

    def _plan_from_widths(self, widths):
        if widths is None or sum(widths) != self.tiles:
            widths = None
        plan, t = [], 0
        if widths is not None:
            for k in widths:
                plan.append((t, k))
                t += k
            return plan
        while t < self.tiles:
            k = min(self.sc, self.tiles - t)
            plan.append((t, k))
            t += k
        return plan

    def chunk_plan_for(self, b):
        # optional taper: narrow first (and last) chunks prime/drain the
        # cross-engine pipeline faster than full-width ones
        full = (self.tiles - 5) // 4
        if self.taper == 1 and self.sc == 4:
            widths = [1, 2] + [4] * full + [2]
        elif self.taper == 2 and self.sc == 4:
            widths = [2] + [4] * full + [3]
        elif self.taper == 3 and self.sc == 4:
            # taper only at body start (b=0) and body end (b=B-1)
            if b == 0:
                widths = [1, 2] + [4] * full + [2]
            else:
                widths = [2] + [4] * full + [2, 1]
        else:
            widths = None
        return self._plan_from_widths(widths)

    @property
    def chunk_plan(self):
        return self.chunk_plan_for(0)

    @property
    def n_chunks(self):
        return len(self.chunk_plan)


def build_nc(cfg: Cfg):
    """Build the per-core Bass program (identical across all 8 cores)."""
    f32 = mybir.dt.float32
    nc = bacc.Bacc("TRN2", target_bir_lowering=False, debug=False)
    T = cfg.tiles
    NCH = cfg.n_chunks

    # host-gathered corners, bf16: [b, chunk, p, t*256 + k*64 + c]
    gc_d = nc.dram_tensor("gcorn", [B, NCH, 128, cfg.sc * 256], BF16,
                          kind="ExternalInput")
    w_d = nc.dram_tensor("wts", [128, T * 4], BF16, kind="ExternalInput")
    mf_d = nc.dram_tensor("mf", [128, T * F], BF16, kind="ExternalInput")
    w1_d = nc.dram_tensor("W1", [DIN, HID], BF16, kind="ExternalInput")
    b1_d = nc.dram_tensor("b1r", [128, 2], f32, kind="ExternalInput")
    # W2 quadrants: [hidhalf*2+outhalf, 128 hid, 128 out]
    w2_d = nc.dram_tensor("W2q", [4, 128, 128], BF16, kind="ExternalInput")
    if cfg.add_b2:
        b2_d = nc.dram_tensor("b2r", [128, 2], f32, kind="ExternalInput")
    # output transposed: [outhalf, outch(128), b*npc + node]
    out_d = nc.dram_tensor("out", [2, 128, B * cfg.npc], BF16,
                           kind="ExternalOutput")

    with tile.TileContext(nc) as tc:
        with (
            tc.tile_pool(name="res", bufs=1) as res,
            tc.tile_pool(name="gp", bufs=cfg.bufs[0]) as gp,
            tc.tile_pool(name="sp", bufs=cfg.bufs[1]) as spool,
            tc.tile_pool(name="xp", bufs=cfg.bufs[2]) as xp,
            tc.tile_pool(name="rp", bufs=4) as rp,
            tc.tile_pool(name="xtp", bufs=cfg.bufs[3]) as xtp,
            tc.tile_pool(name="htp", bufs=cfg.bufs[4]) as htp,
            tc.tile_pool(name="yp", bufs=cfg.bufs[5]) as yp,
            tc.tile_pool(name="ps_xt", bufs=cfg.ps_bufs[0],
                         space="PSUM") as ps_xt,
            tc.tile_pool(name="ps_ht", bufs=cfg.ps_bufs[1],
                         space="PSUM") as ps_ht,
            tc.tile_pool(name="ps_y", bufs=cfg.ps_bufs[2],
                         space="PSUM") as ps_y,
        ):
            w_sb = res.tile([128, T * 4], BF16)
            mf_sb = res.tile([128, T * F], BF16)
            w1_sb = res.tile([DIN, HID], BF16)
            b1_sb = res.tile([128, 2], f32)
            w2_sb = res.tile([128, 4 * 128], BF16)
            ident = res.tile([128, 128], BF16)

            nc.sync.dma_start(out=w_sb[:], in_=w_d[:])
            nc.sync.dma_start(out=mf_sb[:], in_=mf_d[:])
            nc.sync.dma_start(out=w1_sb[:], in_=w1_d[:])
            nc.sync.dma_start(out=b1_sb[:], in_=b1_d[:])
            for q in range(4):
                nc.sync.dma_start(out=w2_sb[:, q * 128:(q + 1) * 128],
                                  in_=w2_d[q])
            if cfg.add_b2:
                b2_sb = res.tile([128, 2], f32)
                nc.sync.dma_start(out=b2_sb[:], in_=b2_d[:])
            make_identity(nc, ident[:])

            def mm_group(gi, src, xoff, nbase, tg0, gt, pe_red=False):
                """4-tile MM group: transpose -> W1 -> relu -> W2 -> store.

                pe_red: src is `scaled` [128, sc*256]; the 4-corner sum runs
                on the PE as 4 accumulating transposes per tile. Otherwise
                src is the DVE-reduced x [128, sc*64], one transpose per tile.
                """
                nn = gt * 128
                xt_ps = ps_xt.tile([DIN, 4 * 128], BF16, tag="xtps")
                for tl in range(gt):
                    if pe_red:
                        for k in range(4):
                            col = ((xoff + tl) * 4 + k) * 64
                            nc.tensor.matmul(
                                out=xt_ps[0:64, tl * 128:(tl + 1) * 128],
                                lhsT=src[:, col:col + 64],
                                rhs=ident[:],
                                is_transpose=True,
                                start=(k == 0), stop=(k == 3),
                            )
                    else:
                        nc.tensor.transpose(
                            out=xt_ps[0:64, tl * 128:(tl + 1) * 128],
                            in_=src[:, (xoff + tl) * 64:(xoff + tl + 1) * 64],
                            identity=ident[:],
                        )
                    nc.tensor.transpose(
                        out=xt_ps[64:68, tl * 128:(tl + 1) * 128],
                        in_=mf_sb[:, (tg0 + tl) * 4:(tg0 + tl + 1) * 4],
                        identity=ident[:],
                    )
                xt = xtp.tile([DIN, 4 * 128], BF16, tag="xt")
                if gi % 2 == 0 or cfg.relu_split:
                    nc.scalar.activation(
                        out=xt[:, :nn], in_=xt_ps[:, :nn],
                        func=mybir.ActivationFunctionType.Copy)
                else:
                    nc.vector.tensor_copy(out=xt[:, :nn], in_=xt_ps[:, :nn])
                # --- layer 1: ht[h, n] = W1h^T @ xt ---
                ht = htp.tile([128, 2 * 512], BF16, tag="ht")
                if cfg.merged_ht and not cfg.add_b1:
                    ht_ps = ps_ht.tile([128, 1024], f32, tag="htps")
                    for h in range(2):
                        nc.tensor.matmul(
                            out=ht_ps[:, h * 512: h * 512 + nn],
                            lhsT=w1_sb[:, h * 128:(h + 1) * 128],
                            rhs=xt[:, :nn], start=True, stop=True,
                        )
                    # single relu over both hid halves (b1 known zero)
                    nc.scalar.activation(
                        out=ht[:].rearrange("p (o n) -> p o n",
                                            o=2)[:, :, :nn],
                        in_=ht_ps[:].rearrange("p (o n) -> p o n",
                                               o=2)[:, :, :nn],
                        func=mybir.ActivationFunctionType.Relu,
                    )
                else:
                    for h in range(2):
                        ht_ps = ps_ht.tile([128, 512], f32, tag="htps")
                        nc.tensor.matmul(
                            out=ht_ps[:, :nn],
                            lhsT=w1_sb[:, h * 128:(h + 1) * 128],
                            rhs=xt[:, :nn], start=True, stop=True,
                        )
                        if cfg.relu_split and h == 1:
                            nc.vector.tensor_scalar(
                                out=ht[:, h * 512: h * 512 + nn],
                                in0=ht_ps[:, :nn],
                                scalar1=b1_sb[:, h:h + 1], scalar2=0.0,
                                op0=mybir.AluOpType.add,
                                op1=mybir.AluOpType.max,
                            )
                        else:
                            nc.scalar.activation(
                                out=ht[:, h * 512: h * 512 + nn],
                                in_=ht_ps[:, :nn],
                                func=mybir.ActivationFunctionType.Relu,
                                bias=b1_sb[:, h:h + 1], scale=1.0,
                            )
                # --- layer 2 transposed: y[o, n] = sum_h W2q^T @ ht ---
                y = yp.tile([128, 2 * 512], BF16, tag="y")
                if cfg.merged_y and not cfg.add_b2:
                    y_ps = ps_y.tile([128, 1024], f32, tag="yps")
                    for oh in range(2):
                        for hh in range(2):
                            nc.tensor.matmul(
                                out=y_ps[:, oh * 512: oh * 512 + nn],
                                lhsT=w2_sb[:, (hh * 2 + oh) * 128:
                                           (hh * 2 + oh + 1) * 128],
                                rhs=ht[:, hh * 512: hh * 512 + nn],
                                start=(hh == 0), stop=(hh == 1),
                            )
                    yv = y[:].rearrange("p (o n) -> p o n", o=2)[:, :, :nn]
                    ypv = y_ps[:].rearrange("p (o n) -> p o n",
                                            o=2)[:, :, :nn]
                    if gi % 2 == 0:
                        nc.scalar.activation(
                            out=yv, in_=ypv,
                            func=mybir.ActivationFunctionType.Copy)
                    else:
                        nc.vector.tensor_copy(out=yv, in_=ypv)
                else:
                    for oh in range(2):
                        y_ps = ps_y.tile([128, 512], f32, tag="yps")
                        for hh in range(2):
                            nc.tensor.matmul(
                                out=y_ps[:, :nn],
                                lhsT=w2_sb[:, (hh * 2 + oh) * 128:
                                           (hh * 2 + oh + 1) * 128],
                                rhs=ht[:, hh * 512: hh * 512 + nn],
                                start=(hh == 0), stop=(hh == 1),
                            )
                        # (oh ^ gi%2) alternation keeps DVE/ACT evenly fed
                        if cfg.add_b2:
                            nc.scalar.activation(
                                out=y[:, oh * 512: oh * 512 + nn],
                                in_=y_ps[:, :nn],
                                func=mybir.ActivationFunctionType.Identity,
                                bias=b2_sb[:, oh:oh + 1], scale=1.0,
                            )
                        elif (oh + gi) % 2 == 0:
                            nc.scalar.activation(
                                out=y[:, oh * 512: oh * 512 + nn],
                                in_=y_ps[:, :nn],
                                func=mybir.ActivationFunctionType.Copy,
                            )
                        else:
                            nc.vector.tensor_copy(
                                out=y[:, oh * 512: oh * 512 + nn],
                                in_=y_ps[:, :nn],
                            )
                if cfg.do_store:
                    n0 = nbase + tg0 * 128
                    eng0 = nc.sync if cfg.stores_split else nc.gpsimd
                    eng1 = nc.gpsimd
                    if cfg.swap_stores:
                        eng0, eng1 = eng1, eng0
                    eng0.dma_start(out=out_d[0, :, n0:n0 + nn],
                                   in_=y[:, 0:nn])
                    eng1.dma_start(out=out_d[1, :, n0:n0 + nn],
                                   in_=y[:, 512: 512 + nn])

            def body():
                gi = 0
                if cfg.interleave_b:
                    order = [(b, ci, t0, kt)
                             for ci, (t0, kt) in enumerate(cfg.chunk_plan)
                             for b in range(B)]
                else:
                    order = [(b, ci, t0, kt)
                             for b in range(B)
                             for ci, (t0, kt) in
                             enumerate(cfg.chunk_plan_for(b))]
                for b, ci, t0, kt in order:
                    if True:
                        # --- dense load of host-gathered corners (bf16) ---
                        if cfg.do_load or cfg.do_mult:
                            g = gp.tile([128, cfg.sc * 256], BF16, tag="g")
                        if cfg.do_load:
                            nc.sync.dma_start(out=g[:, :kt * 256],
                                              in_=gc_d[b, ci, :, :kt * 256])
                        elif cfg.do_mult:
                            nc.gpsimd.memset(g[:, :kt * 256], 0.25)
                        # --- weighted corners: scaled = g * w ---
                        if cfg.do_mult or cfg.do_reduce:
                            scaled = spool.tile([128, cfg.sc * 256], BF16,
                                                tag="s")
                        if cfg.do_mult:
                            if cfg.tck_layout:
                                g_v = g[:, :kt * 256].rearrange(
                                    "p (t c k) -> p t c k", c=64, k=4)
                                w_v = (
                                    w_sb[:, t0 * 4:(t0 + kt) * 4]
                                    .rearrange("p (t k o) -> p t o k",
                                               k=4, o=1)
                                    .to_broadcast([128, kt, 64, 4])
                                )
                                s_v = scaled[:, :kt * 256].rearrange(
                                    "p (t c k) -> p t c k", c=64, k=4)
                            else:
                                g_v = g[:, :kt * 256].rearrange(
                                    "p (t k c) -> p t k c", k=4, c=64)
                                w_v = (
                                    w_sb[:, t0 * 4:(t0 + kt) * 4]
                                    .rearrange("p (t k o) -> p t k o",
                                               k=4, o=1)
                                    .to_broadcast([128, kt, 4, 64])
                                )
                                s_v = scaled[:, :kt * 256].rearrange(
                                    "p (t k c) -> p t k c", k=4, c=64)
                            mult_eng = (nc.vector if cfg.mult_on_dve
                                        else nc.gpsimd)
                            mult_eng.tensor_tensor(out=s_v, in0=g_v, in1=w_v,
                                                   op=mybir.AluOpType.mult)
                        elif cfg.do_reduce:
                            nc.gpsimd.memset(scaled[:, :kt * 256], 0.25)
                        # --- corner sum -> x [128, kt*64] (bf16) ---
                        pe_red = cfg.pe_reduce == 2 or (
                            cfg.pe_reduce == 1 and ci % 2 == 1)
                        if pe_red:
                            src = scaled
                        else:
                            x = xp.tile([128, cfg.sc * 64], BF16, tag="x")
                            src = x
                            if not cfg.do_reduce and cfg.do_mm:
                                nc.gpsimd.memset(x[:, :kt * 64], 0.25)
                            if cfg.do_reduce and cfg.tck_layout:
                                with nc.allow_low_precision(
                                        reason="4-term bf16 sum, tol 2e-2"):
                                    nc.vector.tensor_reduce(
                                        out=x[:, :kt * 64].rearrange(
                                            "p (t c) -> p t c", c=64),
                                        in_=scaled[:, :kt * 256].rearrange(
                                            "p (t c k) -> p t c k",
                                            c=64, k=4),
                                        axis=mybir.AxisListType.X,
                                        op=mybir.AluOpType.add,
                                    )
                            elif cfg.do_reduce and cfg.pool_reduce:
                                s4 = scaled[:, :kt * 256].rearrange(
                                    "p (t k c) -> p t k c", k=4, c=64)
                                ra = rp.tile([128, cfg.sc * 64], BF16,
                                             tag="ra")
                                rb = rp.tile([128, cfg.sc * 64], BF16,
                                             tag="rb")
                                ra_v = ra[:, :kt * 64].rearrange(
                                    "p (t c) -> p t c", c=64)
                                rb_v = rb[:, :kt * 64].rearrange(
                                    "p (t c) -> p t c", c=64)
                                x_v = x[:, :kt * 64].rearrange(
                                    "p (t c) -> p t c", c=64)
                                nc.gpsimd.tensor_tensor(
                                    out=ra_v, in0=s4[:, :, 0, :],
                                    in1=s4[:, :, 1, :],
                                    op=mybir.AluOpType.add)
                                nc.gpsimd.tensor_tensor(
                                    out=rb_v, in0=s4[:, :, 2, :],
                                    in1=s4[:, :, 3, :],
                                    op=mybir.AluOpType.add)
                                nc.gpsimd.tensor_tensor(
                                    out=x_v, in0=ra_v, in1=rb_v,
                                    op=mybir.AluOpType.add)
                            elif cfg.do_reduce:
                                with nc.allow_low_precision(
                                        reason="4-term bf16 sum, tol 2e-2"):
                                    nc.vector.tensor_reduce(
                                        out=x[:, :kt * 64].rearrange(
                                            "p (t c) -> p t c", c=64),
                                        in_=scaled[:, :kt * 256].rearrange(
                                            "p (t k c) -> p t c k",
                                            k=4, c=64),
                                        axis=mybir.AxisListType.X,
                                        op=mybir.AluOpType.add,
                                    )
                        if cfg.do_mm:
                            for g0 in range(0, kt, 4):
                                gt = min(4, kt - g0)
                                mm_group(gi, src, g0, b * cfg.npc,
                                         t0 + g0, gt, pe_red)
                                gi += 1

            if cfg.loop_k > 0:
                with tc.For_i(0, cfg.loop_k, 1,
                              staggered_reset=cfg.staggered):
                    body()
            else:
                body()
    nc.compile()
    return nc


# ---------------------------------------------------------------------------
# Host side
# ---------------------------------------------------------------------------

_NC_CACHE = {}


def _get_nc(cfg: Cfg):
    key = (cfg.add_b2, cfg.npc, cfg.loop_k)
    if key not in _NC_CACHE:
        _NC_CACHE[key] = build_nc(cfg)
    return _NC_CACHE[key]


def _core_layout(arr, npc, core, width):
    """arr: (M_pad, width) -> per-core [128, tiles*width] partition-major."""
    t = npc // 128
    a = arr[core * npc:(core + 1) * npc]
    return np.ascontiguousarray(
        a.reshape(t, 128, width).transpose(1, 0, 2).reshape(128, t * width)
    )


def make_in_maps(grid_data, mesh_features, indices, weights, W1, b1, W2, b2,
                 cfg):
    grid_data = np.asarray(grid_data, dtype=np.float32)
    mesh_features = np.asarray(mesh_features, dtype=np.float32)
    indices = np.asarray(indices).astype(np.int64)
    weights = np.asarray(weights, dtype=np.float32)
    npc = cfg.npc
    m_pad = N_CORES * npc
    T = cfg.tiles

    grid2d = grid_data.reshape(B * G, C).astype(NPDT)

    wp = np.zeros((m_pad, 4), dtype=np.float32)
    wp[:M] = weights
    mfp = np.zeros((m_pad, F), dtype=np.float32)
    mfp[:M] = mesh_features
    idxp = np.zeros((m_pad, 4), dtype=np.int64)
    idxp[:M] = indices

    b1r = np.ascontiguousarray(np.asarray(b1, np.float32).reshape(2, 128).T)
    # W2 quadrants [hh*2+oh, 128, 128]
    w2 = np.asarray(W2, np.float32)
    w2q = np.stack([w2[hh * 128:(hh + 1) * 128, oh * 128:(oh + 1) * 128]
                    for hh in range(2) for oh in range(2)]).astype(NPDT)
    b2r = np.ascontiguousarray(
        np.asarray(b2, np.float32).reshape(2, 128).T)

    in_maps = []
    for c in range(N_CORES):
        idx_c = idxp[c * npc:(c + 1) * npc]  # (npc, 4)
        gcorn = np.zeros((B, cfg.n_chunks, 128, cfg.sc * 256), dtype=NPDT)
        for b in range(B):
            # (npc, 4, C) -> tiles (T,128,4,C) -> (128, T, 4*C)
            g4 = grid2d[b * G + idx_c]
            if cfg.tck_layout:  # -> (npc, C, 4): k innermost
                g4 = np.ascontiguousarray(g4.transpose(0, 2, 1))
            g4 = g4.reshape(T, 128, 4 * C).transpose(1, 0, 2)
            for ci, (t0, kt) in enumerate(cfg.chunk_plan_for(b)):
                gcorn[b, ci, :, :kt * 256] = (
                    g4[:, t0:t0 + kt].reshape(128, kt * 256))
        im = {
            "gcorn": gcorn,
            "wts": _core_layout(wp, npc, c, 4).astype(NPDT),
            "mf": _core_layout(mfp, npc, c, F).astype(NPDT),
            "W1": np.asarray(W1, np.float32).astype(NPDT),
            "b1r": b1r,
            "W2q": w2q,
        }
        if cfg.add_b2:
            im["b2r"] = b2r
        in_maps.append(im)
    return in_maps


def kernel(grid_data, mesh_features, indices, weights, W1, b1, W2, b2):
    cfg = Cfg(add_b2=bool(np.any(np.asarray(b2))),
              add_b1=bool(np.any(np.asarray(b1))))
    nc = _get_nc(cfg)
    in_maps = make_in_maps(grid_data, mesh_features, indices, weights,
                           W1, b1, W2, b2, cfg)

    from concourse.bass_utils import run_bass_kernel_spmd
    res = run_bass_kernel_spmd(nc, in_maps, core_ids=list(range(N_CORES)))

    npc = cfg.npc
    # per-core out: [2(outhalf), 128, B*npc] bf16, nodes ordered [b, node]
    shards = []
    for c in range(N_CORES):
        o = np.asarray(res.results[c]["out"])  # (2, 128, B*npc)
        o = o.reshape(2, 128, B, npc).transpose(2, 3, 0, 1)  # (B,npc,2,128)
        shards.append(o.reshape(B, npc, OUT))
    y = np.concatenate(shards, axis=1)[:, :M, :].astype(np.float32)
    return np.ascontiguousarray(y)
